# revision 1
# baseline (speedup 1.0000x reference)
"""Criss-Cross Attention (CCA) Trainium2 Bass kernel.

Problem: n=8 images of (c=512, h=128, w=128); per-pixel projections
q,k (64ch) and v (512ch); row + column attention with joint softmax over
the 256 (w + h) logits per pixel (self pixel masked out of the column
branch); out = gamma * att + x.

Sharding: data-parallel over batch — one image per NeuronCore (8 cores).

Per-core pipeline (all internal compute fp16 with fp32 PSUM accumulation):
  P1: stream x in 4-row blocks; project q,k (channel-major, fp16 SBUF
      resident) and v (channel-major fp16 -> DRAM scratch).
  P2: e_row/e_col matmuls twice: a sum pass accumulating Z = sum(exp(e))
      per pixel (no max subtraction: |e| <~ 50 so exp stays in fp32
      range), then an output pass computing a = exp(e - ln(Z/gamma))
      (normalized, gamma-folded, <= 1 so fp16-safe), written through a
      ring buffer and batch-transposed (xbar DMA) into [key, out] layout.
  P3: per 128-channel block: v read back via batched transpose DMA into
      row-pixel-major layout; column-pixel-major layout built with PE
      transposes; row/col attention matmuls accumulate channel-major
      output; residual x added exactly in fp32.
"""

import sys

for _p in ("/opt/trn_rl_repo",):
    if _p not in sys.path:
        sys.path.insert(0, _p)

from contextlib import ExitStack

import numpy as np

from concourse import bacc
import concourse.bass as bass
import concourse.mybir as mybir
import concourse.tile as tile
from concourse.bass_utils import run_bass_kernel_spmd

F32 = mybir.dt.float32
F16 = mybir.dt.float16
AX = mybir.AxisListType
ALU = mybir.AluOpType
AF = mybir.ActivationFunctionType

N_CORES = 8
C, H, W = 512, 128, 128
CQK = 64
KC = 4  # input-channel chunks of 128
OC = 4  # output-channel chunks of 128
NEG_INF = -1e9


def build(n_cores: int = N_CORES, dbg: bool = False):
    nc = bacc.Bacc("TRN2", debug=False, num_devices=n_cores)

    x_d = nc.dram_tensor("x", [C, H, W], F32, kind="ExternalInput")
    wq_d = nc.dram_tensor("Wq", [CQK, C], F32, kind="ExternalInput")
    bq_d = nc.dram_tensor("bq", [CQK], F32, kind="ExternalInput")
    wk_d = nc.dram_tensor("Wk", [CQK, C], F32, kind="ExternalInput")
    bk_d = nc.dram_tensor("bk", [CQK], F32, kind="ExternalInput")
    wv_d = nc.dram_tensor("Wv", [C, C], F32, kind="ExternalInput")
    bv_d = nc.dram_tensor("bv", [C], F32, kind="ExternalInput")
    g_d = nc.dram_tensor("gamma", [1], F32, kind="ExternalInput")
    out_d = nc.dram_tensor("out", [C, H, W], F32, kind="ExternalOutput")

    v_scr = nc.dram_tensor(
        "v_scr", [H, C, W], F16, kind="ExternalOutput" if dbg else "Internal"
    )
    nb_scr = nc.dram_tensor("nb_scr", [2, H * W], F16)
    if dbg:
        dbg_outs = {
            "dq": nc.dram_tensor("dq", [CQK, H, W], F16, kind="ExternalOutput"),
            "dk": nc.dram_tensor("dk", [CQK, H, W], F16, kind="ExternalOutput"),
            "ds1": nc.dram_tensor("ds1", [128, H], F32, kind="ExternalOutput"),
            "ds2": nc.dram_tensor("ds2", [128, W], F32, kind="ExternalOutput"),
            "dnbr": nc.dram_tensor("dnbr", [128, H], F32, kind="ExternalOutput"),
            "dnbc": nc.dram_tensor("dnbc", [128, W], F32, kind="ExternalOutput"),
            "dart": nc.dram_tensor(
                "dart", [128, H, 128], F16, kind="ExternalOutput"
            ),
            "dact": nc.dram_tensor(
                "dact", [128, W, 128], F16, kind="ExternalOutput"
            ),
            "dacc": nc.dram_tensor(
                "dacc", [128, H, W], F16, kind="ExternalOutput"
            ),
        }

    with tile.TileContext(nc) as tc, ExitStack() as ctx:
        const = ctx.enter_context(tc.tile_pool(name="const", bufs=1))
        stats = ctx.enter_context(tc.tile_pool(name="stats", bufs=1))

        # ---- constants ----------------------------------------------------
        ident32 = const.tile([128, 128], F32)
        from concourse.masks import make_identity

        make_identity(nc, ident32)
        ident16 = const.tile([128, 128], F16)
        nc.vector.tensor_copy(ident16, ident32)

        diag_neg = const.tile([128, 128], F32)
        nc.gpsimd.memset(diag_neg, 0.0)
        nc.gpsimd.affine_select(
            out=diag_neg,
            in_=diag_neg,
            compare_op=ALU.not_equal,
            fill=NEG_INF,
            base=0,
            pattern=[[-1, 128]],
            channel_multiplier=1,
        )

        diag_neg4 = const.tile([128, 4, 128], F32)
        nc.gpsimd.memset(diag_neg4, 0.0)
        nc.gpsimd.affine_select(
            out=diag_neg4,
            in_=diag_neg4,
            compare_op=ALU.not_equal,
            fill=NEG_INF,
            base=0,
            pattern=[[0, 4], [-1, 128]],
            channel_multiplier=1,
        )

        bq_sb = const.tile([CQK, 1], F32)
        nc.sync.dma_start(out=bq_sb, in_=bq_d[:].rearrange("(a b) -> a b", b=1))
        bk_sb = const.tile([CQK, 1], F32)
        nc.sync.dma_start(out=bk_sb, in_=bk_d[:].rearrange("(a b) -> a b", b=1))
        bv_sb = const.tile([128, OC], F32)
        nc.sync.dma_start(
            out=bv_sb, in_=bv_d[:].rearrange("(o p) -> p o", p=128)
        )
        g_ap = g_d[:]
        g_bcast = bass.AP(
            tensor=g_ap.tensor, offset=g_ap.offset, ap=[[0, 128], [1, 1]]
        )
        g_sb = const.tile([128, 1], F32)
        nc.gpsimd.dma_start(out=g_sb, in_=g_bcast)
        lng = stats.tile([128, 1], F32)
        nc.scalar.activation(lng, g_sb, AF.Ln)

        # transposed projection weights (fp16): wqkT [128, KC, 128] where
        # columns 0:64 = Wq^T chunk, 64:128 = Wk^T chunk; wvT [128, KC, 512]
        wqkT = const.tile([128, KC, 128], F16)
        wvT = const.tile([128, KC, C], F16)
        with tc.tile_pool(name="wprep", bufs=2) as wprep, tc.tile_pool(
            name="wps", bufs=2, space="PSUM"
        ) as wps:
            for kc in range(KC):
                for w_d, col0 in ((wq_d, 0), (wk_d, CQK)):
                    raw = wprep.tile([CQK, 128], F32, tag="rawqk")
                    nc.sync.dma_start(
                        out=raw, in_=w_d[:, kc * 128 : (kc + 1) * 128]
                    )
                    tps = wps.tile([128, CQK], F32, tag="tqk")
                    nc.tensor.transpose(tps, raw, ident32[:CQK, :CQK])
                    nc.vector.tensor_copy(
                        wqkT[:, kc, col0 : col0 + CQK], tps
                    )
                for oc in range(OC):
                    rawv = wprep.tile([128, 128], F32, tag="rawv")
                    nc.sync.dma_start(
                        out=rawv,
                        in_=wv_d[
                            oc * 128 : (oc + 1) * 128, kc * 128 : (kc + 1) * 128
                        ],
                    )
                    tps2 = wps.tile([128, 128], F32, tag="tv")
                    nc.tensor.transpose(tps2, rawv, ident32)
                    nc.vector.tensor_copy(
                        wvT[:, kc, oc * 128 : (oc + 1) * 128], tps2
                    )

        # ---- persistent activation maps ----------------------------------
        # opened before qk so the stack allocator can reclaim qk and the P1/P2
        # transients before P3 pools open.
        a_rowT = ctx.enter_context(tc.tile_pool(name="a_rowT", bufs=1))
        a_colT = ctx.enter_context(tc.tile_pool(name="a_colT", bufs=1))
        a_rowT_t = a_rowT.tile([128, H, 128], F16)  # (v, y, x_out)
        a_colT_t = a_colT.tile([128, W, 128], F16)  # (g, x, y_out)

        s1 = stats.tile([128, H], F32)  # [x, y] row-branch exp sums
        s2 = stats.tile([128, W], F32)  # [y, x] col-branch exp sums
        nb_row = stats.tile([128, H], F32)  # [x, y] = -ln(Z/gamma)^T
        nb_col = stats.tile([128, W], F32)  # [y, x] = -ln(Z/gamma)

        # ==================================================================
        # P1 + P2 transients in a nested scope (freed before P3)
        # ==================================================================
        with ExitStack() as p12:
            qk = p12.enter_context(tc.tile_pool(name="qk", bufs=1))
            # rows 0:64 = channels; row 64,65 = bias hi/lo (q) and ones (k)
            q_sb = qk.tile([CQK + 2, H, W], F16)  # (c, y, x)
            k_sb = qk.tile([CQK + 2, H, W], F16)
            nc.gpsimd.memset(q_sb[CQK : CQK + 2, :, :], 0.0)
            nc.gpsimd.memset(k_sb[CQK : CQK + 2, :, :], 1.0)

            # ---------------- P1: projections -----------------------------
            # groups of 8 4-row blocks; kc-outer so each projection weight
            # chunk is loaded once per group and streams 8 N=512 matmuls.
            GB = 6
            with tc.tile_pool(name="xin", bufs=2) as xin, tc.tile_pool(
                name="x16", bufs=GB + 2
            ) as x16p, tc.tile_pool(name="v16", bufs=4) as v16p, tc.tile_pool(
                name="p1ps", bufs=1, space="PSUM"
            ) as p1ps:
                n_blocks = H // 4
                grp_sizes = [GB] * (n_blocks // GB)
                if n_blocks % GB:
                    grp_sizes.append(n_blocks % GB)
                b0 = 0
                for grp, gsz in enumerate(grp_sizes):
                    x16s = []
                    for b in range(gsz):
                        y0 = 4 * (b0 + b)
                        xt = xin.tile([128, KC, 512], F32, tag="xt")
                        for kc in range(KC):
                            nc.sync.dma_start(
                                out=xt[:, kc, :],
                                in_=x_d[
                                    kc * 128 : (kc + 1) * 128, y0 : y0 + 4, :
                                ].rearrange("c r w -> c (r w)"),
                            )
                        x16 = x16p.tile([128, KC, 512], F16, tag="x16")
                        nc.scalar.copy(
                            x16.rearrange("c k w -> c (k w)"),
                            xt.rearrange("c k w -> c (k w)"),
                        )
                        x16s.append(x16)

                    # q,k: one M=128 matmul per (kc, block), kc-inner
                    qk_pss = [
                        p1ps.tile(
                            [128, 512], F32, tag="qkps", bufs=2,
                            name=f"qkps_{grp}_{i}",
                        )
                        for i in range(gsz)
                    ]
                    for b in range(gsz):
                        for kc in range(KC):
                            nc.tensor.matmul(
                                qk_pss[b],
                                wqkT[:, kc, :],
                                x16s[b][:, kc, :],
                                start=(kc == 0),
                                stop=(kc == KC - 1),
                            )
                    for b in range(gsz):
                        y0 = 4 * (b0 + b)
                        nc.vector.tensor_scalar_add(
                            q_sb[0:CQK, y0 : y0 + 4, :].rearrange(
                                "c r w -> c (r w)"
                            ),
                            qk_pss[b][0:CQK, :],
                            bq_sb,
                        )
                        nc.vector.tensor_scalar_add(
                            k_sb[0:CQK, y0 : y0 + 4, :].rearrange(
                                "c r w -> c (r w)"
                            ),
                            qk_pss[b][CQK:128, :],
                            bk_sb,
                        )

                    for oc in range(OC):
                        v_pss = [
                            p1ps.tile(
                                [128, 512], F32, tag="vps", bufs=6,
                                name=f"vps_{grp}_{oc}_{i}",
                            )
                            for i in range(gsz)
                        ]
                        for kc in range(KC):
                            for b in range(gsz):
                                nc.tensor.matmul(
                                    v_pss[b],
                                    wvT[:, kc, oc * 128 : (oc + 1) * 128],
                                    x16s[b][:, kc, :],
                                    start=(kc == 0),
                                    stop=(kc == KC - 1),
                                )
                        for b in range(gsz):
                            y0 = 4 * (b0 + b)
                            v16 = v16p.tile([128, 512], F16, tag="v16")
                            nc.vector.tensor_scalar_add(
                                v16, v_pss[b], bv_sb[:, oc : oc + 1]
                            )
                            nc.sync.dma_start(
                                out=v_scr[
                                    y0 : y0 + 4, oc * 128 : (oc + 1) * 128, :
                                ].rearrange("r c w -> c r w"),
                                in_=v16.rearrange("c (r w) -> c r w", w=128),
                            )
                    b0 += gsz

            # ---------------- P2: softmax statistics ----------------------
            trash = p12.enter_context(tc.tile_pool(name="trash", bufs=4))
            emsk = p12.enter_context(tc.tile_pool(name="emsk", bufs=4))
            ring = p12.enter_context(tc.tile_pool(name="ring", bufs=2))

            with tc.tile_pool(name="p2ps", bufs=1, space="PSUM") as p2ps:
                # ---- sum pass (no max subtraction; exp stays in fp32 range;
                # q bias rows are still zero here) -------------------------
                for y0 in range(0, H, 4):
                    e_ps = p2ps.tile([128, 4, 128], F32, tag="e_ps", bufs=4)
                    for j in range(4):
                        nc.tensor.matmul(
                            e_ps[:, j, :],
                            q_sb[:, y0 + j, :],
                            k_sb[:, y0 + j, :],
                            start=True,
                            stop=True,
                        )
                    tr = trash.tile([128, 4, 128], F32, tag="trash")
                    nc.scalar.activation(
                        tr.rearrange("p a b -> p (a b)"),
                        e_ps.rearrange("p a b -> p (a b)"),
                        AF.Exp,
                    )
                    nc.vector.reduce_sum(
                        s1[:, y0 : y0 + 4], tr, axis=AX.X
                    )
                for x0 in range(0, W, 4):
                    e_ps = p2ps.tile([128, 4, 128], F32, tag="e_ps", bufs=4)
                    for j in range(4):
                        nc.tensor.matmul(
                            e_ps[:, j, :],
                            q_sb[:, :, x0 + j],
                            k_sb[:, :, x0 + j],
                            start=True,
                            stop=True,
                        )
                    em = emsk.tile([128, 4, 128], F32, tag="emsk")
                    nc.vector.tensor_tensor(
                        em.rearrange("p a b -> p (a b)"),
                        e_ps.rearrange("p a b -> p (a b)"),
                        diag_neg4.rearrange("p a b -> p (a b)"),
                        ALU.add,
                    )
                    tr = trash.tile([128, 4, 128], F32, tag="trash")
                    nc.scalar.activation(
                        tr.rearrange("p a b -> p (a b)"),
                        em.rearrange("p a b -> p (a b)"),
                        AF.Exp,
                    )
                    nc.vector.reduce_sum(
                        s2[:, x0 : x0 + 4], tr, axis=AX.X
                    )

                # ---- nb[y,x] = -(ln(Z) - ln(gamma)); ln via exponent
                # extraction so any fp32 Z is in the ACT Ln table range ----
                zt_ps = p2ps.tile([128, 128], F32, tag="zt", bufs=1)
                nc.tensor.transpose(zt_ps, s1, ident32)
                z_yx = stats.tile([128, W], F32)
                nc.vector.tensor_tensor(z_yx, zt_ps, s2, ALU.add)
                z_i = z_yx[...].bitcast(mybir.dt.int32)
                e_i32 = stats.tile([128, W], mybir.dt.int32)
                nc.vector.tensor_scalar(
                    out=e_i32,
                    in0=z_i,
                    scalar1=23,
                    scalar2=None,
                    op0=ALU.logical_shift_right,
                )
                ef = stats.tile([128, W], F32)
                nc.vector.tensor_scalar(
                    out=ef,
                    in0=e_i32,
                    scalar1=127,
                    scalar2=None,
                    op0=ALU.subtract,
                )
                mant = stats.tile([128, W], F32)
                nc.vector.tensor_scalar(
                    out=mant[...].bitcast(mybir.dt.int32),
                    in0=z_i,
                    scalar1=0x007FFFFF,
                    scalar2=0x3F800000,
                    op0=ALU.bitwise_and,
                    op1=ALU.bitwise_or,
                )
                lnm = stats.tile([128, W], F32)
                nc.scalar.activation(lnm, mant, AF.Ln)
                lnz = stats.tile([128, W], F32)
                nc.vector.scalar_tensor_tensor(
                    out=lnz,
                    in0=ef,
                    scalar=float(np.log(2.0)),
                    in1=lnm,
                    op0=ALU.mult,
                    op1=ALU.add,
                )
                nb_yx = stats.tile([128, W], F32)
                nc.vector.tensor_scalar(
                    out=nb_yx,
                    in0=lnz,
                    scalar1=lng,
                    scalar2=-1.0,
                    op0=ALU.subtract,
                    op1=ALU.mult,
                )
                # hi/lo fp16 split, bounced through DRAM into the two
                # augmented q partitions: e' = e + nb_hi + nb_lo
                nbh = stats.tile([128, W], F16)
                nc.vector.tensor_copy(nbh, nb_yx)
                nbh32 = stats.tile([128, W], F32)
                nc.vector.tensor_copy(nbh32, nbh)
                nbl = stats.tile([128, W], F16)
                nc.vector.tensor_tensor(nbl, nb_yx, nbh32, ALU.subtract)
                nc.sync.dma_start(
                    out=nb_scr[0:1, :].rearrange("o (y x) -> (o y) x", x=W),
                    in_=nbh,
                )
                nc.sync.dma_start(
                    out=nb_scr[1:2, :].rearrange("o (y x) -> (o y) x", x=W),
                    in_=nbl,
                )
                nc.sync.dma_start(
                    out=q_sb[CQK : CQK + 2, :, :].rearrange(
                        "c y x -> c (y x)"
                    ),
                    in_=nb_scr[:, :],
                )

                # ---- a passes: a = exp(e + nb) (normalized, gamma folded),
                # through ring buffers + batched xbar transpose ------------
                for ycb in range(H // 16):
                    rt = ring.tile([128, 16, 128], F16, tag="ring")
                    for j4 in range(4):
                        y0 = ycb * 16 + j4 * 4
                        e_ps = p2ps.tile(
                            [128, 4, 128], F32, tag="e_ps", bufs=4
                        )
                        for j in range(4):
                            nc.tensor.matmul(
                                e_ps[:, j, :],
                                q_sb[:, y0 + j, :],
                                k_sb[:, y0 + j, :],
                                start=True,
                                stop=True,
                            )
                        nc.scalar.activation(
                            rt[:, j4 * 4 : j4 * 4 + 4, :].rearrange(
                                "p a b -> p (a b)"
                            ),
                            e_ps.rearrange("p a b -> p (a b)"),
                            AF.Exp,
                        )
                    nc.sync.dma_start(
                        out=a_rowT_t[:, ycb * 16 : (ycb + 1) * 16, :],
                        in_=rt.rearrange("p a b -> p (a b)"),
                        transpose=True,
                    )
                for xcb in range(W // 16):
                    rt = ring.tile([128, 16, 128], F16, tag="ring")
                    for j4 in range(4):
                        x0 = xcb * 16 + j4 * 4
                        e_ps = p2ps.tile(
                            [128, 4, 128], F32, tag="e_ps", bufs=4
                        )
                        for j in range(4):
                            nc.tensor.matmul(
                                e_ps[:, j, :],
                                q_sb[:, :, x0 + j],
                                k_sb[:, :, x0 + j],
                                start=True,
                                stop=True,
                            )
                        em = emsk.tile([128, 4, 128], F32, tag="emsk")
                        nc.vector.tensor_tensor(
                            em.rearrange("p a b -> p (a b)"),
                            e_ps.rearrange("p a b -> p (a b)"),
                            diag_neg4.rearrange("p a b -> p (a b)"),
                            ALU.add,
                        )
                        nc.scalar.activation(
                            rt[:, j4 * 4 : j4 * 4 + 4, :].rearrange(
                                "p a b -> p (a b)"
                            ),
                            em.rearrange("p a b -> p (a b)"),
                            AF.Exp,
                        )
                    nc.sync.dma_start(
                        out=a_colT_t[:, xcb * 16 : (xcb + 1) * 16, :],
                        in_=rt.rearrange("p a b -> p (a b)"),
                        transpose=True,
                    )

            if dbg:
                for name, src in (
                    ("dq", q_sb),
                    ("dk", k_sb),
                    ("ds1", s1),
                    ("ds2", s2),
                    ("dart", a_rowT_t),
                    ("dact", a_colT_t),
                ):
                    d = dbg_outs[name]
                    nc.sync.dma_start(
                        out=d[...].rearrange(
                            " ".join(f"d{i}" for i in range(len(d.shape)))
                            + " -> d0 ("
                            + " ".join(f"d{i}" for i in range(1, len(d.shape)))
                            + ")"
                        )
                        if len(d.shape) > 2
                        else d[...],
                        in_=src.rearrange("p a b -> p (a b)")
                        if len(src.shape) == 3
                        else src,
                    )

        # ==================================================================
        # P3: attention application, channel-major output
        # ==================================================================
        with ExitStack() as p3:
            vrow = p3.enter_context(tc.tile_pool(name="vrow", bufs=1))
            vcol = p3.enter_context(tc.tile_pool(name="vcol", bufs=1))
            accp = p3.enter_context(tc.tile_pool(name="accp", bufs=1))
            xres = p3.enter_context(tc.tile_pool(name="xres", bufs=3))
            outp = p3.enter_context(tc.tile_pool(name="outp", bufs=3))
            tmpp = p3.enter_context(tc.tile_pool(name="tmpp", bufs=4))

            with tc.tile_pool(name="p3ps", bufs=1, space="PSUM") as p3ps:
                for oc in range(OC):
                    # vrow_t[xv, c, y] = v[c, y, xv] via batched xbar DMA-T
                    vrow_t = vrow.tile([128, 128, H], F16, tag="vrow")
                    for cq in range(4):
                        nc.scalar.dma_start(
                            out=vrow_t[:, cq * 32 : (cq + 1) * 32, :],
                            in_=v_scr[
                                :, oc * 128 + cq * 32 : oc * 128 + (cq + 1) * 32, :
                            ].rearrange("y c x -> y (c x)"),
                            transpose=True,
                        )
                    # vcol_t[g, c, x] = vrow_t[x, c, g] via SBUF xbar DMA-T
                    vcol_t = vcol.tile([128, 128, W], F16, tag="vcol")
                    for cq in range(4):
                        nc.scalar.dma_start(
                            out=vcol_t[:, cq * 32 : (cq + 1) * 32, :],
                            in_=vrow_t[
                                :, cq * 32 : (cq + 1) * 32, :
                            ].rearrange("x c y -> x (c y)"),
                            transpose=True,
                        )
                    acc = accp.tile([128, H, W], F16, tag="acc")  # (c, y, x)
                    for x0 in range(0, W, 4):
                        oc_ps = p3ps.tile([128, 4, 128], F32, tag="ocps", bufs=2)
                        for i in range(4):
                            nc.tensor.matmul(
                                oc_ps[:, i, :],
                                vcol_t[:, :, x0 + i],
                                a_colT_t[:, x0 + i, :],
                                start=True,
                                stop=True,
                            )
                        if (x0 // 4) % 2 == 0:
                            nc.vector.tensor_copy(
                                acc[:, :, x0 : x0 + 4].rearrange(
                                    "c y x -> c x y"
                                ),
                                oc_ps,
                            )
                        else:
                            nc.scalar.copy(
                                acc[:, :, x0 : x0 + 4].rearrange(
                                    "c y x -> c x y"
                                ),
                                oc_ps,
                            )
                    if dbg and oc == 0:
                        nc.sync.dma_start(
                            out=dbg_outs["dacc"][...].rearrange(
                                "c y x -> c (y x)"
                            ),
                            in_=acc.rearrange("c y x -> c (y x)"),
                        )

                    for b in range(H // 4):
                        y0 = 4 * b
                        xr = xres.tile([128, 4, 128], F32, tag="xr")
                        nc.scalar.dma_start(
                            out=xr.rearrange("c r w -> c (r w)"),
                            in_=x_d[
                                oc * 128 : (oc + 1) * 128, y0 : y0 + 4, :
                            ].rearrange("c r w -> c (r w)"),
                        )
                        or_ps = p3ps.tile([128, 4, 128], F32, tag="orps", bufs=2)
                        for j in range(4):
                            nc.tensor.matmul(
                                or_ps[:, j, :],
                                vrow_t[:, :, y0 + j],
                                a_rowT_t[:, y0 + j, :],
                                start=True,
                                stop=True,
                            )
                        tmp = tmpp.tile([128, 4, 128], F32, tag="tmp")
                        nc.vector.tensor_tensor(
                            tmp.rearrange("c r w -> c (r w)"),
                            or_ps.rearrange("c r w -> c (r w)"),
                            acc[:, y0 : y0 + 4, :].rearrange("c r w -> c (r w)"),
                            ALU.add,
                        )
                        ot = outp.tile([128, 4, 128], F32, tag="ot")
                        nc.vector.tensor_tensor(
                            ot.rearrange("c r w -> c (r w)"),
                            tmp.rearrange("c r w -> c (r w)"),
                            xr.rearrange("c r w -> c (r w)"),
                            ALU.add,
                        )
                        nc.sync.dma_start(
                            out=out_d[
                                oc * 128 : (oc + 1) * 128, y0 : y0 + 4, :
                            ].rearrange("c r w -> c (r w)"),
                            in_=ot.rearrange("p a b -> p (a b)"),
                        )

    nc.finalize()
    return nc


_NC_CACHE = {}


def _get_nc():
    if "nc" not in _NC_CACHE:
        _NC_CACHE["nc"] = build()
    return _NC_CACHE["nc"]


def kernel(**inputs) -> np.ndarray:
    x = np.ascontiguousarray(np.asarray(inputs["x"], dtype=np.float32))
    n = x.shape[0]
    assert x.shape == (n, C, H, W)
    shared = {
        name: np.ascontiguousarray(np.asarray(inputs[name], dtype=np.float32))
        for name in ("Wq", "bq", "Wk", "bk", "Wv", "bv", "gamma")
    }
    nc = _get_nc()
    in_maps = [{"x": x[i], **shared} for i in range(n)]
    res = run_bass_kernel_spmd(nc, in_maps, core_ids=list(range(n)))
    return np.stack([res.results[i]["out"] for i in range(n)], axis=0)


if __name__ == "__main__":
    rng = np.random.default_rng(0)
    demo = {
        "x": rng.standard_normal((N_CORES, C, H, W), dtype=np.float32),
        "Wq": rng.standard_normal((CQK, C), dtype=np.float32) / np.sqrt(C),
        "bq": np.zeros(CQK, np.float32),
        "Wk": rng.standard_normal((CQK, C), dtype=np.float32) / np.sqrt(C),
        "bk": np.zeros(CQK, np.float32),
        "Wv": rng.standard_normal((C, C), dtype=np.float32) / np.sqrt(C),
        "bv": np.zeros(C, np.float32),
        "gamma": np.ones(1, np.float32),
    }
    out = kernel(**demo)
    print("out", out.shape, out.dtype, np.abs(out).mean())



# revision 14
# speedup vs baseline: 1.4985x; 1.4985x over previous
"""Criss-Cross Attention (CCA) Trainium2 Bass kernel — v2.

Problem: n=8 images of (c=512, h=128, w=128); per-pixel projections
q,k (64ch) and v (512ch); row + column attention with joint softmax over
the 256 (w + h) logits per pixel (self pixel masked out of the column
branch); out = gamma * att + x.

Sharding: data-parallel over batch — one image per NeuronCore (8 cores).

v2 design notes (all transposes eliminated):
  P1: stream x in 4-row blocks; project q,k channel-major into SBUF;
      project v PIXEL-major (x16 row-slice as matmul stationary, WvT as
      moving) giving [x, c512] tiles, stored to DRAM as v_scr[y][x][c].
  P2: sum pass (query-on-partition e-matmuls, exp, reduce) -> Z;
      nb = -(lnZ - ln gamma) folded into fp16 hi/lo aug rows of q_sb.
      a-pass with SWAPPED operands (k stationary, q moving) so the
      attention maps come out KEY-on-partition and are written straight
      to SBUF — no XBAR transpose DMAs.
  P3: per oc-pair: col branch (v column tiles read from v_scr as 512B
      lines, v-stationary matmuls) -> contiguous fp16 acc [c,x,y];
      row branch matmuls accumulate into PSUM, the col acc is folded in
      by an identity-matmul whose strided MOVING operand does the
      (x<->y) relabel for free on the PE; one fused DVE op adds the
      residual x and gamma*bv, then a straight DMA writes out.
"""

import sys

for _p in ("/opt/trn_rl_repo",):
    if _p not in sys.path:
        sys.path.insert(0, _p)

from contextlib import ExitStack

import numpy as np

from concourse import bacc
import concourse.bass as bass
import concourse.mybir as mybir
import concourse.tile as tile
from concourse.bass_utils import run_bass_kernel_spmd

F32 = mybir.dt.float32
F16 = mybir.dt.float16
AX = mybir.AxisListType
ALU = mybir.AluOpType
AF = mybir.ActivationFunctionType

N_CORES = 8
C, H, W = 512, 128, 128
CQK = 64
KC = 4  # input-channel chunks of 128
OC = 4  # output-channel chunks of 128
NEG_INF = -1e9


def _dap(t, offset, dims):
    """Raw DRAM access pattern: dims = [(stride, count), ...] in elements."""
    a = t[...]
    return bass.AP(
        tensor=a.tensor, offset=a.offset + offset, ap=[[s, n] for s, n in dims]
    )


def build(n_cores: int = N_CORES, dbg: bool = False):
    nc = bacc.Bacc("TRN2", debug=False, num_devices=n_cores)

    x_d = nc.dram_tensor("x", [C, H, W], F32, kind="ExternalInput")
    wq_d = nc.dram_tensor("Wq", [CQK, C], F32, kind="ExternalInput")
    bq_d = nc.dram_tensor("bq", [CQK], F32, kind="ExternalInput")
    wk_d = nc.dram_tensor("Wk", [CQK, C], F32, kind="ExternalInput")
    bk_d = nc.dram_tensor("bk", [CQK], F32, kind="ExternalInput")
    wv_d = nc.dram_tensor("Wv", [C, C], F32, kind="ExternalInput")
    bv_d = nc.dram_tensor("bv", [C], F32, kind="ExternalInput")
    g_d = nc.dram_tensor("gamma", [1], F32, kind="ExternalInput")
    out_d = nc.dram_tensor("out", [C, H, W], F32, kind="ExternalOutput")

    v_scr = nc.dram_tensor(
        "v_scr", [H, W, C], F16, kind="ExternalOutput" if dbg else "Internal"
    )  # pixel-major
    nb_scr = nc.dram_tensor("nb_scr", [2, H * W], F16)
    if dbg:
        dbg_outs = {
            "dq": nc.dram_tensor("dq", [CQK + 2, H, W], F16, kind="ExternalOutput"),
            "dk": nc.dram_tensor("dk", [CQK + 2, H, W], F16, kind="ExternalOutput"),
            "ds1": nc.dram_tensor("ds1", [128, H], F32, kind="ExternalOutput"),
            "ds2": nc.dram_tensor("ds2", [128, W], F32, kind="ExternalOutput"),
            "dart": nc.dram_tensor("dart", [128, H, 128], F16, kind="ExternalOutput"),
            "dact": nc.dram_tensor("dact", [128, W, 128], F16, kind="ExternalOutput"),
            "dacc0": nc.dram_tensor("dacc0", [128, W, H], F16, kind="ExternalOutput"),
        }

    with tile.TileContext(nc) as tc, ExitStack() as ctx:
        const = ctx.enter_context(tc.tile_pool(name="const", bufs=1))
        stats = ctx.enter_context(tc.tile_pool(name="stats", bufs=1))

        # ---- constants ----------------------------------------------------
        ident32 = const.tile([128, 128], F32)
        from concourse.masks import make_identity

        make_identity(nc, ident32)
        ident16 = const.tile([128, 128], F16)
        nc.vector.tensor_copy(ident16, ident32)

        diag_neg4 = const.tile([128, 4, 128], F32)
        nc.gpsimd.memset(diag_neg4, 0.0)
        nc.gpsimd.affine_select(
            out=diag_neg4,
            in_=diag_neg4,
            compare_op=ALU.not_equal,
            fill=NEG_INF,
            base=0,
            pattern=[[0, 4], [-1, 128]],
            channel_multiplier=1,
        )

        bq_sb = const.tile([CQK, 1], F32)
        nc.sync.dma_start(out=bq_sb, in_=bq_d[:].rearrange("(a b) -> a b", b=1))
        bk_sb = const.tile([CQK, 1], F32)
        nc.sync.dma_start(out=bk_sb, in_=bk_d[:].rearrange("(a b) -> a b", b=1))
        bv_sb = const.tile([128, OC], F32)
        nc.sync.dma_start(
            out=bv_sb, in_=bv_d[:].rearrange("(o p) -> p o", p=128)
        )
        g_ap = g_d[:]
        g_bcast = bass.AP(
            tensor=g_ap.tensor, offset=g_ap.offset, ap=[[0, 128], [1, 1]]
        )
        g_sb = const.tile([128, 1], F32)
        nc.gpsimd.dma_start(out=g_sb, in_=g_bcast)
        lng = stats.tile([128, 1], F32)
        nc.scalar.activation(lng, g_sb, AF.Ln)
        # gamma * bv, added in the final residual op
        gbv_sb = const.tile([128, OC], F32)
        nc.vector.tensor_scalar(
            out=gbv_sb, in0=bv_sb, scalar1=g_sb, scalar2=None, op0=ALU.mult
        )

        # transposed projection weights (fp16): wqkT [128, KC, 128] where
        # columns 0:64 = Wq^T chunk, 64:128 = Wk^T chunk; wvT [128, KC, 512]
        wqkT = const.tile([128, KC, 128], F16)
        wvT = const.tile([128, KC, C], F16)
        with tc.tile_pool(name="wprep", bufs=2) as wprep, tc.tile_pool(
            name="wps", bufs=2, space="PSUM"
        ) as wps:
            for kc in range(KC):
                for w_d, col0 in ((wq_d, 0), (wk_d, CQK)):
                    raw = wprep.tile([CQK, 128], F32, tag="rawqk")
                    nc.sync.dma_start(
                        out=raw, in_=w_d[:, kc * 128 : (kc + 1) * 128]
                    )
                    tps = wps.tile([128, CQK], F32, tag="tqk")
                    nc.tensor.transpose(tps, raw, ident32[:CQK, :CQK])
                    nc.vector.tensor_copy(
                        wqkT[:, kc, col0 : col0 + CQK], tps
                    )
                for oc in range(OC):
                    rawv = wprep.tile([128, 128], F32, tag="rawv")
                    nc.sync.dma_start(
                        out=rawv,
                        in_=wv_d[
                            oc * 128 : (oc + 1) * 128, kc * 128 : (kc + 1) * 128
                        ],
                    )
                    tps2 = wps.tile([128, 128], F32, tag="tv")
                    nc.tensor.transpose(tps2, rawv, ident32)
                    nc.vector.tensor_copy(
                        wvT[:, kc, oc * 128 : (oc + 1) * 128], tps2
                    )

        # ---- persistent attention maps (key-on-partition, fp16) ----------
        a_rowT = ctx.enter_context(tc.tile_pool(name="a_rowT", bufs=1))
        a_colT = ctx.enter_context(tc.tile_pool(name="a_colT", bufs=1))
        a_rowT_t = a_rowT.tile([128, H, 128], F16)  # (xk, y, xq)
        a_colT_t = a_colT.tile([128, W, 128], F16)  # (g,  x, yq)

        s1 = stats.tile([128, H], F32)  # [xq, y] row-branch exp sums
        s2 = stats.tile([128, W], F32)  # [yq, x] col-branch exp sums

        # ==================================================================
        # P1 + P2 in a nested scope so q/k free their SBUF before P3
        # ==================================================================
        with ExitStack() as p12:
            qk = p12.enter_context(tc.tile_pool(name="qk", bufs=1))
            # rows 0:64 = channels; rows 64,65 = nb hi/lo (q) and ones (k)
            q_sb = qk.tile([CQK + 2, H, W], F16)  # (c, y, x)
            k_sb = qk.tile([CQK + 2, H, W], F16)
            nc.gpsimd.memset(q_sb[CQK : CQK + 2, :, :], 0.0)
            nc.gpsimd.memset(k_sb[CQK : CQK + 2, :, :], 1.0)

            # ---------------- P1: projections -----------------------------
            with tc.tile_pool(name="xin", bufs=3) as xin, tc.tile_pool(
                name="x16", bufs=3
            ) as x16p, tc.tile_pool(name="v16", bufs=2) as v16p, tc.tile_pool(
                name="p1ps", bufs=1, space="PSUM"
            ) as p1ps:
                for b in range(H // 4):
                    y0 = 4 * b
                    xt = xin.tile([128, KC, 512], F32, tag="xt")
                    for kc in range(KC):
                        nc.sync.dma_start(
                            out=xt[:, kc, :],
                            in_=x_d[
                                kc * 128 : (kc + 1) * 128, y0 : y0 + 4, :
                            ].rearrange("c r w -> c (r w)"),
                        )
                    x16 = x16p.tile([128, KC, 512], F16, tag="x16")
                    cast = nc.scalar.copy if b % 2 == 0 else nc.vector.tensor_copy
                    cast(
                        x16.rearrange("c k w -> c (k w)"),
                        xt.rearrange("c k w -> c (k w)"),
                    )

                    # q,k channel-major: psum [qk128, (4y,128x)]
                    qk_ps = p1ps.tile([128, 512], F32, tag="qkps", bufs=2)
                    for kc in range(KC):
                        nc.tensor.matmul(
                            qk_ps,
                            wqkT[:, kc, :],
                            x16[:, kc, :],
                            start=(kc == 0),
                            stop=(kc == KC - 1),
                        )
                    nc.vector.tensor_scalar_add(
                        q_sb[0:CQK, y0 : y0 + 4, :].rearrange(
                            "c r w -> c (r w)"
                        ),
                        qk_ps[0:CQK, :],
                        bq_sb,
                    )
                    nc.vector.tensor_scalar_add(
                        k_sb[0:CQK, y0 : y0 + 4, :].rearrange(
                            "c r w -> c (r w)"
                        ),
                        qk_ps[CQK:128, :],
                        bk_sb,
                    )

                    # v pixel-major: per row y, psum [x, c512]
                    v16 = v16p.tile([128, 4, C], F16, tag="v16")
                    for j in range(4):
                        v_ps = p1ps.tile([128, C], F32, tag="vps", bufs=4)
                        for kc in range(KC):
                            nc.tensor.matmul(
                                v_ps,
                                x16[:, kc, j * 128 : (j + 1) * 128],
                                wvT[:, kc, :],
                                start=(kc == 0),
                                stop=(kc == KC - 1),
                            )
                        vcp = (
                            nc.scalar.copy if j % 2 == 0 else nc.vector.tensor_copy
                        )
                        vcp(v16[:, j, :], v_ps)
                    nc.sync.dma_start(
                        out=_dap(
                            v_scr,
                            y0 * W * C,
                            [(C, 128), (W * C, 4), (1, C)],
                        ),
                        in_=v16,
                    )

            # ---------------- P2: softmax statistics ----------------------
            trash = p12.enter_context(tc.tile_pool(name="trash", bufs=4))
            emsk = p12.enter_context(tc.tile_pool(name="emsk", bufs=4))

            with tc.tile_pool(name="p2ps", bufs=1, space="PSUM") as p2ps:
                # ---- sum pass (query on partitions; q aug rows still 0) --
                for y0 in range(0, H, 4):
                    e_ps = p2ps.tile([128, 4, 128], F32, tag="e_ps", bufs=4)
                    for j in range(4):
                        nc.tensor.matmul(
                            e_ps[:, j, :],
                            q_sb[:, y0 + j, :],
                            k_sb[:, y0 + j, :],
                            start=True,
                            stop=True,
                        )
                    tr = trash.tile([128, 4, 128], F32, tag="trash")
                    nc.scalar.activation(
                        tr.rearrange("p a b -> p (a b)"),
                        e_ps.rearrange("p a b -> p (a b)"),
                        AF.Exp,
                    )
                    nc.vector.reduce_sum(s1[:, y0 : y0 + 4], tr, axis=AX.X)
                for x0 in range(0, W, 4):
                    e_ps = p2ps.tile([128, 4, 128], F32, tag="e_ps", bufs=4)
                    for j in range(4):
                        nc.tensor.matmul(
                            e_ps[:, j, :],
                            q_sb[:, :, x0 + j],
                            k_sb[:, :, x0 + j],
                            start=True,
                            stop=True,
                        )
                    em = emsk.tile([128, 4, 128], F32, tag="emsk")
                    nc.vector.tensor_tensor(
                        em.rearrange("p a b -> p (a b)"),
                        e_ps.rearrange("p a b -> p (a b)"),
                        diag_neg4.rearrange("p a b -> p (a b)"),
                        ALU.add,
                    )
                    tr = trash.tile([128, 4, 128], F32, tag="trash")
                    nc.scalar.activation(
                        tr.rearrange("p a b -> p (a b)"),
                        em.rearrange("p a b -> p (a b)"),
                        AF.Exp,
                    )
                    nc.vector.reduce_sum(s2[:, x0 : x0 + 4], tr, axis=AX.X)

                # ---- nb[y,x] = -(ln(Z) - ln(gamma)); ln via exponent
                # extraction so any fp32 Z is in the ACT Ln table range ----
                zt_ps = p2ps.tile([128, 128], F32, tag="zt", bufs=1)
                nc.tensor.transpose(zt_ps, s1, ident32)
                z_yx = stats.tile([128, W], F32)
                nc.vector.tensor_tensor(z_yx, zt_ps, s2, ALU.add)
                z_i = z_yx[...].bitcast(mybir.dt.int32)
                e_i32 = stats.tile([128, W], mybir.dt.int32)
                nc.vector.tensor_scalar(
                    out=e_i32,
                    in0=z_i,
                    scalar1=23,
                    scalar2=None,
                    op0=ALU.logical_shift_right,
                )
                ef = stats.tile([128, W], F32)
                nc.vector.tensor_scalar(
                    out=ef,
                    in0=e_i32,
                    scalar1=127,
                    scalar2=None,
                    op0=ALU.subtract,
                )
                mant = stats.tile([128, W], F32)
                nc.vector.tensor_scalar(
                    out=mant[...].bitcast(mybir.dt.int32),
                    in0=z_i,
                    scalar1=0x007FFFFF,
                    scalar2=0x3F800000,
                    op0=ALU.bitwise_and,
                    op1=ALU.bitwise_or,
                )
                lnm = stats.tile([128, W], F32)
                nc.scalar.activation(lnm, mant, AF.Ln)
                lnz = stats.tile([128, W], F32)
                nc.vector.scalar_tensor_tensor(
                    out=lnz,
                    in0=ef,
                    scalar=float(np.log(2.0)),
                    in1=lnm,
                    op0=ALU.mult,
                    op1=ALU.add,
                )
                nb_yx = stats.tile([128, W], F32)
                nc.vector.tensor_scalar(
                    out=nb_yx,
                    in0=lnz,
                    scalar1=lng,
                    scalar2=-1.0,
                    op0=ALU.subtract,
                    op1=ALU.mult,
                )
                # hi/lo fp16 split, bounced through DRAM into the two
                # augmented q partitions: e' = e + nb_hi + nb_lo
                nbh = stats.tile([128, W], F16)
                nc.vector.tensor_copy(nbh, nb_yx)
                nbh32 = stats.tile([128, W], F32)
                nc.vector.tensor_copy(nbh32, nbh)
                nbl = stats.tile([128, W], F16)
                nc.vector.tensor_tensor(nbl, nb_yx, nbh32, ALU.subtract)
                nc.sync.dma_start(
                    out=nb_scr[0:1, :].rearrange("o (y x) -> (o y) x", x=W),
                    in_=nbh,
                )
                nc.sync.dma_start(
                    out=nb_scr[1:2, :].rearrange("o (y x) -> (o y) x", x=W),
                    in_=nbl,
                )
                nc.sync.dma_start(
                    out=q_sb[CQK : CQK + 2, :, :].rearrange(
                        "c y x -> c (y x)"
                    ),
                    in_=nb_scr[:, :],
                )

                # ---- a passes: swapped operands (k stationary, q moving)
                # so psum comes out [key, query]; exp writes maps directly -
                for y0 in range(0, H, 4):
                    e_ps = p2ps.tile([128, 4, 128], F32, tag="e_ps", bufs=4)
                    for j in range(4):
                        nc.tensor.matmul(
                            e_ps[:, j, :],
                            k_sb[:, y0 + j, :],
                            q_sb[:, y0 + j, :],
                            start=True,
                            stop=True,
                        )
                    nc.scalar.activation(
                        a_rowT_t[:, y0 : y0 + 4, :].rearrange(
                            "p a b -> p (a b)"
                        ),
                        e_ps.rearrange("p a b -> p (a b)"),
                        AF.Exp,
                    )
                for x0 in range(0, W, 4):
                    e_ps = p2ps.tile([128, 4, 128], F32, tag="e_ps", bufs=4)
                    for j in range(4):
                        nc.tensor.matmul(
                            e_ps[:, j, :],
                            k_sb[:, :, x0 + j],
                            q_sb[:, :, x0 + j],
                            start=True,
                            stop=True,
                        )
                    em = emsk.tile([128, 4, 128], F32, tag="emsk")
                    nc.vector.tensor_tensor(
                        em.rearrange("p a b -> p (a b)"),
                        e_ps.rearrange("p a b -> p (a b)"),
                        diag_neg4.rearrange("p a b -> p (a b)"),
                        ALU.add,
                    )
                    nc.scalar.activation(
                        a_colT_t[:, x0 : x0 + 4, :].rearrange(
                            "p a b -> p (a b)"
                        ),
                        em.rearrange("p a b -> p (a b)"),
                        AF.Exp,
                    )

            if dbg:
                for name, src in (
                    ("dq", q_sb),
                    ("dk", k_sb),
                    ("ds1", s1),
                    ("ds2", s2),
                    ("dart", a_rowT_t),
                    ("dact", a_colT_t),
                ):
                    d = dbg_outs[name]
                    nc.sync.dma_start(
                        out=d[...].rearrange("a b c -> a (b c)")
                        if len(d.shape) == 3
                        else d[...],
                        in_=src.rearrange("p a b -> p (a b)")
                        if len(src.shape) == 3
                        else src[0 : d.shape[0], :],
                    )

        # ==================================================================
        # P3: attention application, oc-pair at a time
        # ==================================================================
        with ExitStack() as p3:
            accp = p3.enter_context(tc.tile_pool(name="accp", bufs=1))
            vcolp = p3.enter_context(tc.tile_pool(name="vcolp", bufs=3))
            vrowp = p3.enter_context(tc.tile_pool(name="vrowp", bufs=3))
            xres = p3.enter_context(tc.tile_pool(name="xres", bufs=4))
            outp = p3.enter_context(tc.tile_pool(name="outp", bufs=4))

            with tc.tile_pool(name="p3ps", bufs=1, space="PSUM") as p3ps:
                for op in range(OC // 2):  # oc pair
                    oc0 = 2 * op
                    # --- col branch: acc[c', x, y] per oc in pair --------
                    accs = [
                        accp.tile(
                            [128, W, H], F16, tag=f"acc{s}", name=f"acc_{op}_{s}"
                        )
                        for s in range(2)
                    ]
                    for x0 in range(0, W, 4):
                        vc = vcolp.tile([128, 4, 256], F16, tag="vc")
                        nc.sync.dma_start(
                            out=vc,
                            in_=_dap(
                                v_scr,
                                x0 * C + oc0 * 128,
                                [(W * C, 128), (C, 4), (1, 256)],
                            ),
                        )
                        for s in range(2):
                            pc_ps = p3ps.tile(
                                [128, 4, 128], F32, tag="pc", bufs=4
                            )
                            for j in range(4):
                                nc.tensor.matmul(
                                    pc_ps[:, j, :],
                                    vc[:, j, s * 128 : (s + 1) * 128],
                                    a_colT_t[:, x0 + j, :],
                                    start=True,
                                    stop=True,
                                )
                            ccp = (
                                nc.scalar.copy
                                if (x0 // 4 + s) % 2 == 0
                                else nc.vector.tensor_copy
                            )
                            ccp(
                                accs[s][:, x0 : x0 + 4, :].rearrange(
                                    "c x y -> c (x y)"
                                ),
                                pc_ps.rearrange("c x y -> c (x y)"),
                            )

                    if dbg and op == 0:
                        nc.sync.dma_start(
                            out=dbg_outs["dacc0"][...].rearrange(
                                "a b c -> a (b c)"
                            ),
                            in_=accs[0].rearrange("p a b -> p (a b)"),
                        )
                    # --- row branch + combine + residual -----------------
                    for y0 in range(0, H, 4):
                        vr = vrowp.tile([128, 4, 256], F16, tag="vr")
                        nc.sync.dma_start(
                            out=vr,
                            in_=_dap(
                                v_scr,
                                y0 * W * C + oc0 * 128,
                                [(C, 128), (W * C, 4), (1, 256)],
                            ),
                        )
                        for s in range(2):
                            oc = oc0 + s
                            xr = xres.tile([128, 4, 128], F32, tag="xr")
                            nc.gpsimd.dma_start(
                                out=xr.rearrange("c r w -> c (r w)"),
                                in_=x_d[
                                    oc * 128 : (oc + 1) * 128, y0 : y0 + 4, :
                                ].rearrange("c r w -> c (r w)"),
                            )
                            pr_ps = p3ps.tile(
                                [128, 4, 128], F32, tag="pr", bufs=4
                            )
                            # col acc relabel (x<->y) first: whole-tile
                            # identity matmul with strided moving operand
                            acc_ap = accs[s][...]
                            acc_mov = bass.AP(
                                tensor=acc_ap.tensor,
                                offset=acc_ap.offset + y0,
                                ap=[list(acc_ap.ap[0]), [1, 4], [H, W]],
                            )
                            nc.tensor.matmul(
                                pr_ps.rearrange("c r w -> c (r w)"),
                                ident16,
                                acc_mov,
                                start=True,
                                stop=False,
                                skip_group_check=True,
                            )
                            for j in range(4):
                                nc.tensor.matmul(
                                    pr_ps[:, j, :],
                                    vr[:, j, s * 128 : (s + 1) * 128],
                                    a_rowT_t[:, y0 + j, :],
                                    start=False,
                                    stop=(j == 3),
                                    skip_group_check=True,
                                )
                            ot = outp.tile([128, 4, 128], F32, tag="ot")
                            nc.vector.scalar_tensor_tensor(
                                out=ot.rearrange("c r w -> c (r w)"),
                                in0=pr_ps.rearrange("c r w -> c (r w)"),
                                scalar=gbv_sb[:, oc : oc + 1],
                                in1=xr.rearrange("c r w -> c (r w)"),
                                op0=ALU.add,
                                op1=ALU.add,
                            )
                            nc.sync.dma_start(
                                out=out_d[
                                    oc * 128 : (oc + 1) * 128, y0 : y0 + 4, :
                                ].rearrange("c r w -> c (r w)"),
                                in_=ot.rearrange("p a b -> p (a b)"),
                            )

    nc.finalize()
    return nc


_NC_CACHE = {}


def _get_nc():
    if "nc" not in _NC_CACHE:
        _NC_CACHE["nc"] = build()
    return _NC_CACHE["nc"]


def kernel(**inputs) -> np.ndarray:
    x = np.ascontiguousarray(np.asarray(inputs["x"], dtype=np.float32))
    n = x.shape[0]
    assert x.shape == (n, C, H, W)
    shared = {
        name: np.ascontiguousarray(np.asarray(inputs[name], dtype=np.float32))
        for name in ("Wq", "bq", "Wk", "bk", "Wv", "bv", "gamma")
    }
    nc = _get_nc()
    in_maps = [{"x": x[i], **shared} for i in range(n)]
    res = run_bass_kernel_spmd(nc, in_maps, core_ids=list(range(n)))
    return np.stack([res.results[i]["out"] for i in range(n)], axis=0)


if __name__ == "__main__":
    rng = np.random.default_rng(0)
    demo = {
        "x": rng.standard_normal((N_CORES, C, H, W), dtype=np.float32),
        "Wq": rng.standard_normal((CQK, C), dtype=np.float32) / np.sqrt(C),
        "bq": np.zeros(CQK, np.float32),
        "Wk": rng.standard_normal((CQK, C), dtype=np.float32) / np.sqrt(C),
        "bk": np.zeros(CQK, np.float32),
        "Wv": rng.standard_normal((C, C), dtype=np.float32) / np.sqrt(C),
        "bv": np.zeros(C, np.float32),
        "gamma": np.ones(1, np.float32),
    }
    out = kernel(**demo)
    print("out", out.shape, out.dtype, np.abs(out).mean())


# revision 19
# speedup vs baseline: 1.6054x; 1.0713x over previous
"""Criss-Cross Attention (CCA) Trainium2 Bass kernel — v2.

Problem: n=8 images of (c=512, h=128, w=128); per-pixel projections
q,k (64ch) and v (512ch); row + column attention with joint softmax over
the 256 (w + h) logits per pixel (self pixel masked out of the column
branch); out = gamma * att + x.

Sharding: data-parallel over batch — one image per NeuronCore (8 cores).

v2 design notes (all transposes eliminated):
  P1: stream x in 4-row blocks; project q,k channel-major into SBUF;
      project v PIXEL-major (x16 row-slice as matmul stationary, WvT as
      moving) giving [x, c512] tiles, stored to DRAM as v_scr[y][x][c].
  P2: sum pass (query-on-partition e-matmuls, exp, reduce) -> Z;
      nb = -(lnZ - ln gamma) folded into fp16 hi/lo aug rows of q_sb.
      a-pass with SWAPPED operands (k stationary, q moving) so the
      attention maps come out KEY-on-partition and are written straight
      to SBUF — no XBAR transpose DMAs.
  P3: per oc-pair: col branch (v column tiles read from v_scr as 512B
      lines, v-stationary matmuls) -> contiguous fp16 acc [c,x,y];
      row branch matmuls accumulate into PSUM, the col acc is folded in
      by an identity-matmul whose strided MOVING operand does the
      (x<->y) relabel for free on the PE; one fused DVE op adds the
      residual x and gamma*bv, then a straight DMA writes out.
"""

import sys

for _p in ("/opt/trn_rl_repo",):
    if _p not in sys.path:
        sys.path.insert(0, _p)

from contextlib import ExitStack

import numpy as np

from concourse import bacc
import concourse.bass as bass
import concourse.mybir as mybir
import concourse.tile as tile
from concourse.bass_utils import run_bass_kernel_spmd

F32 = mybir.dt.float32
F16 = mybir.dt.float16
AX = mybir.AxisListType
ALU = mybir.AluOpType
AF = mybir.ActivationFunctionType

N_CORES = 8
C, H, W = 512, 128, 128
CQK = 64
KC = 4  # input-channel chunks of 128
OC = 4  # output-channel chunks of 128
NEG_INF = -1e9


def _dap(t, offset, dims):
    """Raw DRAM access pattern: dims = [(stride, count), ...] in elements."""
    a = t[...]
    return bass.AP(
        tensor=a.tensor, offset=a.offset + offset, ap=[[s, n] for s, n in dims]
    )


def build(n_cores: int = N_CORES, dbg: bool = False):
    nc = bacc.Bacc("TRN2", debug=False, num_devices=n_cores)

    x_d = nc.dram_tensor("x", [C, H, W], F32, kind="ExternalInput")
    wq_d = nc.dram_tensor("Wq", [CQK, C], F32, kind="ExternalInput")
    bq_d = nc.dram_tensor("bq", [CQK], F32, kind="ExternalInput")
    wk_d = nc.dram_tensor("Wk", [CQK, C], F32, kind="ExternalInput")
    bk_d = nc.dram_tensor("bk", [CQK], F32, kind="ExternalInput")
    wv_d = nc.dram_tensor("Wv", [C, C], F32, kind="ExternalInput")
    bv_d = nc.dram_tensor("bv", [C], F32, kind="ExternalInput")
    g_d = nc.dram_tensor("gamma", [1], F32, kind="ExternalInput")
    out_d = nc.dram_tensor("out", [C, H, W], F32, kind="ExternalOutput")

    v_scr = nc.dram_tensor(
        "v_scr", [H, W, C], F16, kind="ExternalOutput" if dbg else "Internal"
    )  # pixel-major
    nb_scr = nc.dram_tensor("nb_scr", [2, H * W], F16)
    if dbg:
        dbg_outs = {
            "dq": nc.dram_tensor("dq", [CQK + 2, H, W], F16, kind="ExternalOutput"),
            "dk": nc.dram_tensor("dk", [CQK + 2, H, W], F16, kind="ExternalOutput"),
            "ds1": nc.dram_tensor("ds1", [128, H], F32, kind="ExternalOutput"),
            "ds2": nc.dram_tensor("ds2", [128, W], F32, kind="ExternalOutput"),
            "dart": nc.dram_tensor("dart", [128, H, 128], F16, kind="ExternalOutput"),
            "dact": nc.dram_tensor("dact", [128, W, 128], F16, kind="ExternalOutput"),
            "dacc0": nc.dram_tensor("dacc0", [128, W, H], F16, kind="ExternalOutput"),
        }

    with tile.TileContext(nc) as tc, ExitStack() as ctx:
        const = ctx.enter_context(tc.tile_pool(name="const", bufs=1))
        stats = ctx.enter_context(tc.tile_pool(name="stats", bufs=1))

        # ---- constants ----------------------------------------------------
        ident32 = const.tile([128, 128], F32)
        from concourse.masks import make_identity

        make_identity(nc, ident32)
        ident16 = const.tile([128, 128], F16)
        nc.vector.tensor_copy(ident16, ident32)

        # PE-side diagonal mask: diagS16 (diag = -30000) as stationary,
        # identrep4 (identity tiled 4x along free) as moving; one N=512
        # matmul adds -30000 at (g, (j, g)) into the psum group.
        diagS16 = const.tile([128, 128], F16)
        identrep4 = const.tile([128, 4, 128], F16)
        with tc.tile_pool(name="mprep", bufs=1) as mprep:
            dneg = mprep.tile([128, 128], F32)
            nc.gpsimd.memset(dneg, 0.0)
            nc.gpsimd.affine_select(
                out=dneg,
                in_=dneg,
                compare_op=ALU.not_equal,
                fill=-30000.0,
                base=0,
                pattern=[[-1, 128]],
                channel_multiplier=1,
            )
            nc.vector.tensor_copy(diagS16, dneg)
        for j in range(4):
            nc.vector.tensor_copy(identrep4[:, j, :], ident16)

        bq_sb = const.tile([CQK, 1], F32)
        nc.sync.dma_start(out=bq_sb, in_=bq_d[:].rearrange("(a b) -> a b", b=1))
        bk_sb = const.tile([CQK, 1], F32)
        nc.sync.dma_start(out=bk_sb, in_=bk_d[:].rearrange("(a b) -> a b", b=1))
        bv_sb = const.tile([128, OC], F32)
        nc.sync.dma_start(
            out=bv_sb, in_=bv_d[:].rearrange("(o p) -> p o", p=128)
        )
        g_ap = g_d[:]
        g_bcast = bass.AP(
            tensor=g_ap.tensor, offset=g_ap.offset, ap=[[0, 128], [1, 1]]
        )
        g_sb = const.tile([128, 1], F32)
        nc.gpsimd.dma_start(out=g_sb, in_=g_bcast)
        lng = stats.tile([128, 1], F32)
        nc.scalar.activation(lng, g_sb, AF.Ln)
        # gamma * bv, added in the final residual op
        gbv_sb = const.tile([128, OC], F32)
        nc.vector.tensor_scalar(
            out=gbv_sb, in0=bv_sb, scalar1=g_sb, scalar2=None, op0=ALU.mult
        )

        # transposed projection weights (fp16): wqkT [128, KC, 128] where
        # columns 0:64 = Wq^T chunk, 64:128 = Wk^T chunk; wvT [128, KC, 512]
        wqkT = const.tile([128, KC, 128], F16)
        wvT = const.tile([128, KC, C], F16)
        with tc.tile_pool(name="wprep", bufs=2) as wprep, tc.tile_pool(
            name="wps", bufs=2, space="PSUM"
        ) as wps:
            for kc in range(KC):
                for w_d, col0 in ((wq_d, 0), (wk_d, CQK)):
                    raw = wprep.tile([CQK, 128], F32, tag="rawqk")
                    nc.sync.dma_start(
                        out=raw, in_=w_d[:, kc * 128 : (kc + 1) * 128]
                    )
                    tps = wps.tile([128, CQK], F32, tag="tqk")
                    nc.tensor.transpose(tps, raw, ident32[:CQK, :CQK])
                    nc.vector.tensor_copy(
                        wqkT[:, kc, col0 : col0 + CQK], tps
                    )
                for oc in range(OC):
                    rawv = wprep.tile([128, 128], F32, tag="rawv")
                    nc.sync.dma_start(
                        out=rawv,
                        in_=wv_d[
                            oc * 128 : (oc + 1) * 128, kc * 128 : (kc + 1) * 128
                        ],
                    )
                    tps2 = wps.tile([128, 128], F32, tag="tv")
                    nc.tensor.transpose(tps2, rawv, ident32)
                    nc.vector.tensor_copy(
                        wvT[:, kc, oc * 128 : (oc + 1) * 128], tps2
                    )

        # ---- persistent attention maps (key-on-partition, fp16) ----------
        a_rowT = ctx.enter_context(tc.tile_pool(name="a_rowT", bufs=1))
        a_colT = ctx.enter_context(tc.tile_pool(name="a_colT", bufs=1))
        a_rowT_t = a_rowT.tile([128, H, 128], F16)  # (xk, y, xq)
        a_colT_t = a_colT.tile([128, W, 128], F16)  # (g,  x, yq)

        s1 = stats.tile([128, H], F32)  # [xq, y] row-branch exp sums
        s2 = stats.tile([128, W], F32)  # [yq, x] col-branch exp sums

        # ==================================================================
        # P1 + P2 in a nested scope so q/k free their SBUF before P3
        # ==================================================================
        with ExitStack() as p12:
            qk = p12.enter_context(tc.tile_pool(name="qk", bufs=1))
            # rows 0:64 = channels; rows 64,65 = nb hi/lo (q) and ones (k)
            q_sb = qk.tile([CQK + 2, H, W], F16)  # (c, y, x)
            k_sb = qk.tile([CQK + 2, H, W], F16)
            nc.gpsimd.memset(q_sb[CQK : CQK + 2, :, :], 0.0)
            nc.gpsimd.memset(k_sb[CQK : CQK + 2, :, :], 1.0)

            # ---------------- P1: projections -----------------------------
            with tc.tile_pool(name="xin", bufs=3) as xin, tc.tile_pool(
                name="x16", bufs=3
            ) as x16p, tc.tile_pool(name="v16", bufs=2) as v16p, tc.tile_pool(
                name="p1ps", bufs=1, space="PSUM"
            ) as p1ps:
                for b in range(H // 4):
                    y0 = 4 * b
                    xt = xin.tile([128, KC, 512], F32, tag="xt")
                    for kc in range(KC):
                        nc.sync.dma_start(
                            out=xt[:, kc, :],
                            in_=x_d[
                                kc * 128 : (kc + 1) * 128, y0 : y0 + 4, :
                            ].rearrange("c r w -> c (r w)"),
                        )
                    x16 = x16p.tile([128, KC, 512], F16, tag="x16")
                    cast = nc.scalar.copy if b % 2 == 0 else nc.vector.tensor_copy
                    cast(
                        x16.rearrange("c k w -> c (k w)"),
                        xt.rearrange("c k w -> c (k w)"),
                    )

                    # q,k channel-major: psum [qk128, (4y,128x)]
                    qk_ps = p1ps.tile([128, 512], F32, tag="qkps", bufs=2)
                    for kc in range(KC):
                        nc.tensor.matmul(
                            qk_ps,
                            wqkT[:, kc, :],
                            x16[:, kc, :],
                            start=(kc == 0),
                            stop=(kc == KC - 1),
                        )
                    nc.vector.tensor_scalar_add(
                        q_sb[0:CQK, y0 : y0 + 4, :].rearrange(
                            "c r w -> c (r w)"
                        ),
                        qk_ps[0:CQK, :],
                        bq_sb,
                    )
                    nc.vector.tensor_scalar_add(
                        k_sb[0:CQK, y0 : y0 + 4, :].rearrange(
                            "c r w -> c (r w)"
                        ),
                        qk_ps[CQK:128, :],
                        bk_sb,
                    )

                    # v pixel-major: per row y, psum [x, c512]
                    v16 = v16p.tile([128, 4, C], F16, tag="v16")
                    for j in range(4):
                        v_ps = p1ps.tile([128, C], F32, tag="vps", bufs=4)
                        for kc in range(KC):
                            nc.tensor.matmul(
                                v_ps,
                                x16[:, kc, j * 128 : (j + 1) * 128],
                                wvT[:, kc, :],
                                start=(kc == 0),
                                stop=(kc == KC - 1),
                            )
                        vcp = (
                            nc.scalar.copy if j % 2 == 0 else nc.vector.tensor_copy
                        )
                        vcp(v16[:, j, :], v_ps)
                    nc.sync.dma_start(
                        out=_dap(
                            v_scr,
                            y0 * W * C,
                            [(C, 128), (W * C, 4), (1, C)],
                        ),
                        in_=v16,
                    )

            # ---------------- P2: softmax statistics ----------------------
            trash = p12.enter_context(tc.tile_pool(name="trash", bufs=4))

            with tc.tile_pool(name="p2ps", bufs=1, space="PSUM") as p2ps:
                # ---- sum pass (query on partitions; q aug rows still 0) --
                for y0 in range(0, H, 4):
                    e_ps = p2ps.tile([128, 4, 128], F32, tag="e_ps", bufs=4)
                    for j in range(4):
                        nc.tensor.matmul(
                            e_ps[:, j, :],
                            q_sb[:, y0 + j, :],
                            k_sb[:, y0 + j, :],
                            start=True,
                            stop=True,
                        )
                    tr = trash.tile([128, 4, 128], F32, tag="trash")
                    nc.scalar.activation(
                        tr.rearrange("p a b -> p (a b)"),
                        e_ps.rearrange("p a b -> p (a b)"),
                        AF.Exp,
                    )
                    nc.vector.reduce_sum(s1[:, y0 : y0 + 4], tr, axis=AX.X)
                for x0 in range(0, W, 4):
                    e_ps = p2ps.tile([128, 4, 128], F32, tag="e_ps", bufs=4)
                    nc.tensor.matmul(
                        e_ps.rearrange("p a b -> p (a b)"),
                        diagS16,
                        identrep4.rearrange("p a b -> p (a b)"),
                        start=True,
                        stop=False,
                        skip_group_check=True,
                    )
                    for j in range(4):
                        nc.tensor.matmul(
                            e_ps[:, j, :],
                            q_sb[:, :, x0 + j],
                            k_sb[:, :, x0 + j],
                            start=False,
                            stop=(j == 3),
                            skip_group_check=True,
                        )
                    tr = trash.tile([128, 4, 128], F32, tag="trash")
                    nc.scalar.activation(
                        tr.rearrange("p a b -> p (a b)"),
                        e_ps.rearrange("p a b -> p (a b)"),
                        AF.Exp,
                    )
                    nc.vector.reduce_sum(s2[:, x0 : x0 + 4], tr, axis=AX.X)

                # ---- nb[y,x] = -(ln(Z) - ln(gamma)); ln via exponent
                # extraction so any fp32 Z is in the ACT Ln table range ----
                zt_ps = p2ps.tile([128, 128], F32, tag="zt", bufs=1)
                nc.tensor.transpose(zt_ps, s1, ident32)
                z_yx = stats.tile([128, W], F32)
                nc.vector.tensor_tensor(z_yx, zt_ps, s2, ALU.add)
                z_i = z_yx[...].bitcast(mybir.dt.int32)
                e_i32 = stats.tile([128, W], mybir.dt.int32)
                nc.vector.tensor_scalar(
                    out=e_i32,
                    in0=z_i,
                    scalar1=23,
                    scalar2=None,
                    op0=ALU.logical_shift_right,
                )
                ef = stats.tile([128, W], F32)
                nc.vector.tensor_scalar(
                    out=ef,
                    in0=e_i32,
                    scalar1=127,
                    scalar2=None,
                    op0=ALU.subtract,
                )
                mant = stats.tile([128, W], F32)
                nc.vector.tensor_scalar(
                    out=mant[...].bitcast(mybir.dt.int32),
                    in0=z_i,
                    scalar1=0x007FFFFF,
                    scalar2=0x3F800000,
                    op0=ALU.bitwise_and,
                    op1=ALU.bitwise_or,
                )
                lnm = stats.tile([128, W], F32)
                nc.scalar.activation(lnm, mant, AF.Ln)
                lnz = stats.tile([128, W], F32)
                nc.vector.scalar_tensor_tensor(
                    out=lnz,
                    in0=ef,
                    scalar=float(np.log(2.0)),
                    in1=lnm,
                    op0=ALU.mult,
                    op1=ALU.add,
                )
                nb_yx = stats.tile([128, W], F32)
                nc.vector.tensor_scalar(
                    out=nb_yx,
                    in0=lnz,
                    scalar1=lng,
                    scalar2=-1.0,
                    op0=ALU.subtract,
                    op1=ALU.mult,
                )
                # hi/lo fp16 split, bounced through DRAM into the two
                # augmented q partitions: e' = e + nb_hi + nb_lo
                nbh = stats.tile([128, W], F16)
                nc.vector.tensor_copy(nbh, nb_yx)
                nbh32 = stats.tile([128, W], F32)
                nc.vector.tensor_copy(nbh32, nbh)
                nbl = stats.tile([128, W], F16)
                nc.vector.tensor_tensor(nbl, nb_yx, nbh32, ALU.subtract)
                nc.sync.dma_start(
                    out=nb_scr[0:1, :].rearrange("o (y x) -> (o y) x", x=W),
                    in_=nbh,
                )
                nc.sync.dma_start(
                    out=nb_scr[1:2, :].rearrange("o (y x) -> (o y) x", x=W),
                    in_=nbl,
                )
                nc.sync.dma_start(
                    out=q_sb[CQK : CQK + 2, :, :].rearrange(
                        "c y x -> c (y x)"
                    ),
                    in_=nb_scr[:, :],
                )

                # ---- a passes: swapped operands (k stationary, q moving)
                # so psum comes out [key, query]; exp writes maps directly.
                # col pass first so P3's col branch can start earliest.
                for x0 in range(0, W, 4):
                    e_ps = p2ps.tile([128, 4, 128], F32, tag="e_ps", bufs=4)
                    nc.tensor.matmul(
                        e_ps.rearrange("p a b -> p (a b)"),
                        diagS16,
                        identrep4.rearrange("p a b -> p (a b)"),
                        start=True,
                        stop=False,
                        skip_group_check=True,
                    )
                    for j in range(4):
                        nc.tensor.matmul(
                            e_ps[:, j, :],
                            k_sb[:, :, x0 + j],
                            q_sb[:, :, x0 + j],
                            start=False,
                            stop=(j == 3),
                            skip_group_check=True,
                        )
                    nc.scalar.activation(
                        a_colT_t[:, x0 : x0 + 4, :].rearrange(
                            "p a b -> p (a b)"
                        ),
                        e_ps.rearrange("p a b -> p (a b)"),
                        AF.Exp,
                    )
                for y0 in range(0, H, 4):
                    e_ps = p2ps.tile([128, 4, 128], F32, tag="e_ps", bufs=4)
                    for j in range(4):
                        nc.tensor.matmul(
                            e_ps[:, j, :],
                            k_sb[:, y0 + j, :],
                            q_sb[:, y0 + j, :],
                            start=True,
                            stop=True,
                        )
                    nc.scalar.activation(
                        a_rowT_t[:, y0 : y0 + 4, :].rearrange(
                            "p a b -> p (a b)"
                        ),
                        e_ps.rearrange("p a b -> p (a b)"),
                        AF.Exp,
                    )

            if dbg:
                for name, src in (
                    ("dq", q_sb),
                    ("dk", k_sb),
                    ("ds1", s1),
                    ("ds2", s2),
                    ("dart", a_rowT_t),
                    ("dact", a_colT_t),
                ):
                    d = dbg_outs[name]
                    nc.sync.dma_start(
                        out=d[...].rearrange("a b c -> a (b c)")
                        if len(d.shape) == 3
                        else d[...],
                        in_=src.rearrange("p a b -> p (a b)")
                        if len(src.shape) == 3
                        else src[0 : d.shape[0], :],
                    )

        # ==================================================================
        # P3: attention application, oc-pair at a time
        # ==================================================================
        with ExitStack() as p3:
            accp = p3.enter_context(tc.tile_pool(name="accp", bufs=1))
            vcolp = p3.enter_context(tc.tile_pool(name="vcolp", bufs=8))
            vrowp = p3.enter_context(tc.tile_pool(name="vrowp", bufs=8))
            xres = p3.enter_context(tc.tile_pool(name="xres", bufs=6))
            outp = p3.enter_context(tc.tile_pool(name="outp", bufs=4))

            with tc.tile_pool(name="p3ps", bufs=1, space="PSUM") as p3ps:
                for op in range(OC // 2):  # oc pair
                    oc0 = 2 * op
                    # --- col branch: acc[c', x, y] per oc in pair --------
                    accs = [
                        accp.tile(
                            [128, W, H], F16, tag=f"acc{s}", name=f"acc_{op}_{s}"
                        )
                        for s in range(2)
                    ]
                    for x0 in range(0, W, 4):
                        vc = vcolp.tile([128, 4, 256], F16, tag="vc")
                        nc.sync.dma_start(
                            out=vc,
                            in_=_dap(
                                v_scr,
                                x0 * C + oc0 * 128,
                                [(W * C, 128), (C, 4), (1, 256)],
                            ),
                        )
                        for s in range(2):
                            pc_ps = p3ps.tile(
                                [128, 4, 128], F32, tag="pc", bufs=4
                            )
                            for j in range(4):
                                nc.tensor.matmul(
                                    pc_ps[:, j, :],
                                    vc[:, j, s * 128 : (s + 1) * 128],
                                    a_colT_t[:, x0 + j, :],
                                    start=True,
                                    stop=True,
                                )
                            ccp = (
                                nc.scalar.copy
                                if (x0 // 4 + s) % 2 == 0
                                else nc.vector.tensor_copy
                            )
                            ccp(
                                accs[s][:, x0 : x0 + 4, :].rearrange(
                                    "c x y -> c (x y)"
                                ),
                                pc_ps.rearrange("c x y -> c (x y)"),
                            )

                    if dbg and op == 0:
                        nc.sync.dma_start(
                            out=dbg_outs["dacc0"][...].rearrange(
                                "a b c -> a (b c)"
                            ),
                            in_=accs[0].rearrange("p a b -> p (a b)"),
                        )
                    # --- row branch + combine + residual -----------------
                    for y0 in range(0, H, 4):
                        vr = vrowp.tile([128, 4, 256], F16, tag="vr")
                        nc.sync.dma_start(
                            out=vr,
                            in_=_dap(
                                v_scr,
                                y0 * W * C + oc0 * 128,
                                [(C, 128), (W * C, 4), (1, 256)],
                            ),
                        )
                        for s in range(2):
                            oc = oc0 + s
                            xr = xres.tile([128, 4, 128], F32, tag="xr")
                            nc.gpsimd.dma_start(
                                out=xr.rearrange("c r w -> c (r w)"),
                                in_=x_d[
                                    oc * 128 : (oc + 1) * 128, y0 : y0 + 4, :
                                ].rearrange("c r w -> c (r w)"),
                            )
                            pr_ps = p3ps.tile(
                                [128, 4, 128], F32, tag="pr", bufs=4
                            )
                            # col acc relabel (x<->y) first: whole-tile
                            # identity matmul with strided moving operand
                            acc_ap = accs[s][...]
                            acc_mov = bass.AP(
                                tensor=acc_ap.tensor,
                                offset=acc_ap.offset + y0,
                                ap=[list(acc_ap.ap[0]), [1, 4], [H, W]],
                            )
                            nc.tensor.matmul(
                                pr_ps.rearrange("c r w -> c (r w)"),
                                ident16,
                                acc_mov,
                                start=True,
                                stop=False,
                                skip_group_check=True,
                            )
                            for j in range(4):
                                nc.tensor.matmul(
                                    pr_ps[:, j, :],
                                    vr[:, j, s * 128 : (s + 1) * 128],
                                    a_rowT_t[:, y0 + j, :],
                                    start=False,
                                    stop=(j == 3),
                                    skip_group_check=True,
                                )
                            ot = outp.tile([128, 4, 128], F32, tag="ot")
                            nc.vector.scalar_tensor_tensor(
                                out=ot.rearrange("c r w -> c (r w)"),
                                in0=pr_ps.rearrange("c r w -> c (r w)"),
                                scalar=gbv_sb[:, oc : oc + 1],
                                in1=xr.rearrange("c r w -> c (r w)"),
                                op0=ALU.add,
                                op1=ALU.add,
                            )
                            nc.sync.dma_start(
                                out=out_d[
                                    oc * 128 : (oc + 1) * 128, y0 : y0 + 4, :
                                ].rearrange("c r w -> c (r w)"),
                                in_=ot.rearrange("p a b -> p (a b)"),
                            )

    nc.finalize()
    return nc


_NC_CACHE = {}


def _get_nc():
    if "nc" not in _NC_CACHE:
        _NC_CACHE["nc"] = build()
    return _NC_CACHE["nc"]


def kernel(**inputs) -> np.ndarray:
    x = np.ascontiguousarray(np.asarray(inputs["x"], dtype=np.float32))
    n = x.shape[0]
    assert x.shape == (n, C, H, W)
    shared = {
        name: np.ascontiguousarray(np.asarray(inputs[name], dtype=np.float32))
        for name in ("Wq", "bq", "Wk", "bk", "Wv", "bv", "gamma")
    }
    nc = _get_nc()
    in_maps = [{"x": x[i], **shared} for i in range(n)]
    res = run_bass_kernel_spmd(nc, in_maps, core_ids=list(range(n)))
    return np.stack([res.results[i]["out"] for i in range(n)], axis=0)


if __name__ == "__main__":
    rng = np.random.default_rng(0)
    demo = {
        "x": rng.standard_normal((N_CORES, C, H, W), dtype=np.float32),
        "Wq": rng.standard_normal((CQK, C), dtype=np.float32) / np.sqrt(C),
        "bq": np.zeros(CQK, np.float32),
        "Wk": rng.standard_normal((CQK, C), dtype=np.float32) / np.sqrt(C),
        "bk": np.zeros(CQK, np.float32),
        "Wv": rng.standard_normal((C, C), dtype=np.float32) / np.sqrt(C),
        "bv": np.zeros(C, np.float32),
        "gamma": np.ones(1, np.float32),
    }
    out = kernel(**demo)
    print("out", out.shape, out.dtype, np.abs(out).mean())


# revision 25
# speedup vs baseline: 1.7260x; 1.0751x over previous
"""Criss-Cross Attention (CCA) Trainium2 Bass kernel — v2.

Problem: n=8 images of (c=512, h=128, w=128); per-pixel projections
q,k (64ch) and v (512ch); row + column attention with joint softmax over
the 256 (w + h) logits per pixel (self pixel masked out of the column
branch); out = gamma * att + x.

Sharding: data-parallel over batch — one image per NeuronCore (8 cores).

v2 design notes (all transposes eliminated):
  P1: stream x in 4-row blocks; project q,k channel-major into SBUF;
      project v PIXEL-major (x16 row-slice as matmul stationary, WvT as
      moving) giving [x, c512] tiles, stored to DRAM as v_scr[y][x][c].
  P2: sum pass (query-on-partition e-matmuls, exp, reduce) -> Z;
      nb = -(lnZ - ln gamma) folded into fp16 hi/lo aug rows of q_sb.
      a-pass with SWAPPED operands (k stationary, q moving) so the
      attention maps come out KEY-on-partition and are written straight
      to SBUF — no XBAR transpose DMAs.
  P3: per oc-pair: col branch (v column tiles read from v_scr as 512B
      lines, v-stationary matmuls) -> contiguous fp16 acc [c,x,y];
      row branch matmuls accumulate into PSUM, the col acc is folded in
      by an identity-matmul whose strided MOVING operand does the
      (x<->y) relabel for free on the PE; one fused DVE op adds the
      residual x and gamma*bv, then a straight DMA writes out.
"""

import sys

for _p in ("/opt/trn_rl_repo",):
    if _p not in sys.path:
        sys.path.insert(0, _p)

from contextlib import ExitStack

import numpy as np

from concourse import bacc
import concourse.bass as bass
import concourse.mybir as mybir
import concourse.tile as tile
from concourse.bass_utils import run_bass_kernel_spmd

F32 = mybir.dt.float32
F16 = mybir.dt.float16
AX = mybir.AxisListType
ALU = mybir.AluOpType
AF = mybir.ActivationFunctionType

N_CORES = 8
C, H, W = 512, 128, 128
CQK = 64
KC = 4  # input-channel chunks of 128
OC = 4  # output-channel chunks of 128
NEG_INF = -1e9


def _dap(t, offset, dims):
    """Raw DRAM access pattern: dims = [(stride, count), ...] in elements."""
    a = t[...]
    return bass.AP(
        tensor=a.tensor, offset=a.offset + offset, ap=[[s, n] for s, n in dims]
    )


def build(n_cores: int = N_CORES, dbg: bool = False):
    nc = bacc.Bacc("TRN2", debug=False, num_devices=n_cores)

    x_d = nc.dram_tensor("x", [C, H, W], F32, kind="ExternalInput")
    wq_d = nc.dram_tensor("Wq", [CQK, C], F32, kind="ExternalInput")
    bq_d = nc.dram_tensor("bq", [CQK], F32, kind="ExternalInput")
    wk_d = nc.dram_tensor("Wk", [CQK, C], F32, kind="ExternalInput")
    bk_d = nc.dram_tensor("bk", [CQK], F32, kind="ExternalInput")
    wv_d = nc.dram_tensor("Wv", [C, C], F32, kind="ExternalInput")
    bv_d = nc.dram_tensor("bv", [C], F32, kind="ExternalInput")
    g_d = nc.dram_tensor("gamma", [1], F32, kind="ExternalInput")
    out_d = nc.dram_tensor("out", [C, H, W], F16, kind="ExternalOutput")

    v_scr = nc.dram_tensor(
        "v_scr", [H, W, C], F16, kind="ExternalOutput" if dbg else "Internal"
    )  # pixel-major
    nb_scr = nc.dram_tensor("nb_scr", [2, H * W], F16)
    if dbg:
        dbg_outs = {
            "dq": nc.dram_tensor("dq", [CQK + 2, H, W], F16, kind="ExternalOutput"),
            "dk": nc.dram_tensor("dk", [CQK + 2, H, W], F16, kind="ExternalOutput"),
            "ds1": nc.dram_tensor("ds1", [128, H], F32, kind="ExternalOutput"),
            "ds2": nc.dram_tensor("ds2", [128, W], F32, kind="ExternalOutput"),
            "dart": nc.dram_tensor("dart", [128, H, 128], F16, kind="ExternalOutput"),
            "dact": nc.dram_tensor("dact", [128, W, 128], F16, kind="ExternalOutput"),
            "dacc0": nc.dram_tensor("dacc0", [128, W, H], F16, kind="ExternalOutput"),
        }

    with tile.TileContext(nc) as tc, ExitStack() as ctx:
        const = ctx.enter_context(tc.tile_pool(name="const", bufs=1))
        stats = ctx.enter_context(tc.tile_pool(name="stats", bufs=1))

        # ---- constants ----------------------------------------------------
        ident32 = const.tile([128, 128], F32)
        from concourse.masks import make_identity

        make_identity(nc, ident32)
        ident16 = const.tile([128, 128], F16)
        nc.vector.tensor_copy(ident16, ident32)



        bq_sb = const.tile([CQK, 1], F32)
        nc.sync.dma_start(out=bq_sb, in_=bq_d[:].rearrange("(a b) -> a b", b=1))
        bk_sb = const.tile([CQK, 1], F32)
        nc.sync.dma_start(out=bk_sb, in_=bk_d[:].rearrange("(a b) -> a b", b=1))
        bv_sb = const.tile([128, OC], F32)
        nc.sync.dma_start(
            out=bv_sb, in_=bv_d[:].rearrange("(o p) -> p o", p=128)
        )
        g_ap = g_d[:]
        g_bcast = bass.AP(
            tensor=g_ap.tensor, offset=g_ap.offset, ap=[[0, 128], [1, 1]]
        )
        g_sb = const.tile([128, 1], F32)
        nc.gpsimd.dma_start(out=g_sb, in_=g_bcast)
        lng = stats.tile([128, 1], F32)
        nc.scalar.activation(lng, g_sb, AF.Ln)
        # gamma * bv, added in the final residual op
        gbv_sb = const.tile([128, OC], F32)
        nc.vector.tensor_scalar(
            out=gbv_sb, in0=bv_sb, scalar1=g_sb, scalar2=None, op0=ALU.mult
        )

        # transposed projection weights (fp16): wqkT [128, KC, 128] where
        # columns 0:64 = Wq^T chunk, 64:128 = Wk^T chunk; wvT [128, KC, 512]
        wqkT = const.tile([128, KC, 128], F16)
        wvT = const.tile([128, KC, C], F16)
        with tc.tile_pool(name="wprep", bufs=2) as wprep, tc.tile_pool(
            name="wps", bufs=2, space="PSUM"
        ) as wps:
            for kc in range(KC):
                for w_d, col0 in ((wq_d, 0), (wk_d, CQK)):
                    raw = wprep.tile([CQK, 128], F32, tag="rawqk")
                    nc.sync.dma_start(
                        out=raw, in_=w_d[:, kc * 128 : (kc + 1) * 128]
                    )
                    tps = wps.tile([128, CQK], F32, tag="tqk")
                    nc.tensor.transpose(tps, raw, ident32[:CQK, :CQK])
                    nc.vector.tensor_copy(
                        wqkT[:, kc, col0 : col0 + CQK], tps
                    )
                for oc in range(OC):
                    rawv = wprep.tile([128, 128], F32, tag="rawv")
                    nc.sync.dma_start(
                        out=rawv,
                        in_=wv_d[
                            oc * 128 : (oc + 1) * 128, kc * 128 : (kc + 1) * 128
                        ],
                    )
                    tps2 = wps.tile([128, 128], F32, tag="tv")
                    nc.tensor.transpose(tps2, rawv, ident32)
                    nc.vector.tensor_copy(
                        wvT[:, kc, oc * 128 : (oc + 1) * 128], tps2
                    )

        # ---- persistent attention maps (key-on-partition, fp16) ----------
        a_rowT = ctx.enter_context(tc.tile_pool(name="a_rowT", bufs=1))
        a_colT = ctx.enter_context(tc.tile_pool(name="a_colT", bufs=1))
        a_rowT_t = a_rowT.tile([128, H, 128], F16)  # (xk, y, xq)
        a_colT_t = a_colT.tile([128, W, 128], F16)  # (g,  x, yq)

        s1 = stats.tile([128, H], F32)  # [xq, y] row-branch exp sums
        s2 = stats.tile([128, W], F32)  # [yq, x] col-branch exp sums

        # ==================================================================
        # P1 + P2 in a nested scope so q/k free their SBUF before P3
        # ==================================================================
        with ExitStack() as p12:
            qk = p12.enter_context(tc.tile_pool(name="qk", bufs=1))
            # rows 0:64 = channels; rows 64,65 = nb hi/lo (q) and ones (k)
            q_sb = qk.tile([CQK + 2, H, W], F16)  # (c, y, x)
            k_sb = qk.tile([CQK + 2, H, W], F16)
            nc.gpsimd.memset(q_sb[CQK : CQK + 2, :, :], 0.0)
            nc.gpsimd.memset(k_sb[CQK : CQK + 2, :, :], 1.0)

            # ---------------- P1: projections -----------------------------
            with tc.tile_pool(name="xin", bufs=3) as xin, tc.tile_pool(
                name="x16", bufs=3
            ) as x16p, tc.tile_pool(name="v16", bufs=2) as v16p, tc.tile_pool(
                name="p1ps", bufs=1, space="PSUM"
            ) as p1ps:
                for b in range(H // 4):
                    y0 = 4 * b
                    xt = xin.tile([128, KC, 512], F32, tag="xt")
                    for kc in range(KC):
                        nc.sync.dma_start(
                            out=xt[:, kc, :],
                            in_=x_d[
                                kc * 128 : (kc + 1) * 128, y0 : y0 + 4, :
                            ].rearrange("c r w -> c (r w)"),
                        )
                    x16 = x16p.tile([128, KC, 512], F16, tag="x16")
                    cast = nc.scalar.copy if b % 2 == 0 else nc.vector.tensor_copy
                    cast(
                        x16.rearrange("c k w -> c (k w)"),
                        xt.rearrange("c k w -> c (k w)"),
                    )

                    # q,k channel-major: psum [qk128, (4y,128x)]
                    qk_ps = p1ps.tile([128, 512], F32, tag="qkps", bufs=2)
                    for kc in range(KC):
                        nc.tensor.matmul(
                            qk_ps,
                            wqkT[:, kc, :],
                            x16[:, kc, :],
                            start=(kc == 0),
                            stop=(kc == KC - 1),
                        )
                    nc.vector.tensor_scalar_add(
                        q_sb[0:CQK, y0 : y0 + 4, :].rearrange(
                            "c r w -> c (r w)"
                        ),
                        qk_ps[0:CQK, :],
                        bq_sb,
                    )
                    nc.vector.tensor_scalar_add(
                        k_sb[0:CQK, y0 : y0 + 4, :].rearrange(
                            "c r w -> c (r w)"
                        ),
                        qk_ps[CQK:128, :],
                        bk_sb,
                    )

                    # v pixel-major: per row y, psum [x, c512]
                    v16 = v16p.tile([128, 4, C], F16, tag="v16")
                    for j in range(4):
                        v_ps = p1ps.tile([128, C], F32, tag="vps", bufs=4)
                        for kc in range(KC):
                            nc.tensor.matmul(
                                v_ps,
                                x16[:, kc, j * 128 : (j + 1) * 128],
                                wvT[:, kc, :],
                                start=(kc == 0),
                                stop=(kc == KC - 1),
                            )
                        vcp = (
                            nc.scalar.copy if j % 2 == 0 else nc.vector.tensor_copy
                        )
                        vcp(v16[:, j, :], v_ps)
                    nc.sync.dma_start(
                        out=_dap(
                            v_scr,
                            y0 * W * C,
                            [(C, 128), (W * C, 4), (1, C)],
                        ),
                        in_=v16,
                    )

            # ---------------- P2: softmax statistics ----------------------
            trash = p12.enter_context(tc.tile_pool(name="trash", bufs=4))

            with tc.tile_pool(name="p2ps", bufs=1, space="PSUM") as p2ps:
                # ---- sum pass (query on partitions; q aug rows still 0) --
                for y0 in range(0, H, 4):
                    e_ps = p2ps.tile([128, 4, 128], F32, tag="e_ps", bufs=4)
                    for j in range(4):
                        nc.tensor.matmul(
                            e_ps[:, j, :],
                            q_sb[:, y0 + j, :],
                            k_sb[:, y0 + j, :],
                            start=True,
                            stop=True,
                        )
                    tr = trash.tile([128, 4, 128], F32, tag="trash")
                    nc.scalar.activation(
                        tr.rearrange("p a b -> p (a b)"),
                        e_ps.rearrange("p a b -> p (a b)"),
                        AF.Exp,
                    )
                    nc.vector.reduce_sum(s1[:, y0 : y0 + 4], tr, axis=AX.X)
                for x0 in range(0, W, 4):
                    e_ps = p2ps.tile([128, 4, 128], F32, tag="e_ps", bufs=4)
                    for j in range(4):
                        nc.tensor.matmul(
                            e_ps[:, j, :],
                            q_sb[:, :, x0 + j],
                            k_sb[:, :, x0 + j],
                            start=True,
                            stop=True,
                        )
                    tr = trash.tile([128, 4, 128], F32, tag="trash")
                    nc.scalar.activation(
                        tr.rearrange("p a b -> p (a b)"),
                        e_ps.rearrange("p a b -> p (a b)"),
                        AF.Exp,
                    )
                    # zero the self-pixel (diag yk == yq) before the reduce
                    nc.gpsimd.affine_select(
                        out=tr,
                        in_=tr,
                        compare_op=ALU.not_equal,
                        fill=0.0,
                        base=0,
                        pattern=[[0, 4], [-1, 128]],
                        channel_multiplier=1,
                    )
                    nc.vector.reduce_sum(s2[:, x0 : x0 + 4], tr, axis=AX.X)

                # ---- nb[y,x] = -(ln(Z) - ln(gamma)); ln via exponent
                # extraction so any fp32 Z is in the ACT Ln table range ----
                zt_ps = p2ps.tile([128, 128], F32, tag="zt", bufs=1)
                nc.tensor.transpose(zt_ps, s1, ident32)
                z_yx = stats.tile([128, W], F32)
                nc.vector.tensor_tensor(z_yx, zt_ps, s2, ALU.add)
                z_i = z_yx[...].bitcast(mybir.dt.int32)
                e_i32 = stats.tile([128, W], mybir.dt.int32)
                nc.vector.tensor_scalar(
                    out=e_i32,
                    in0=z_i,
                    scalar1=23,
                    scalar2=None,
                    op0=ALU.logical_shift_right,
                )
                ef = stats.tile([128, W], F32)
                nc.vector.tensor_scalar(
                    out=ef,
                    in0=e_i32,
                    scalar1=127,
                    scalar2=None,
                    op0=ALU.subtract,
                )
                mant = stats.tile([128, W], F32)
                nc.vector.tensor_scalar(
                    out=mant[...].bitcast(mybir.dt.int32),
                    in0=z_i,
                    scalar1=0x007FFFFF,
                    scalar2=0x3F800000,
                    op0=ALU.bitwise_and,
                    op1=ALU.bitwise_or,
                )
                lnm = stats.tile([128, W], F32)
                nc.scalar.activation(lnm, mant, AF.Ln)
                lnz = stats.tile([128, W], F32)
                nc.vector.scalar_tensor_tensor(
                    out=lnz,
                    in0=ef,
                    scalar=float(np.log(2.0)),
                    in1=lnm,
                    op0=ALU.mult,
                    op1=ALU.add,
                )
                nb_yx = stats.tile([128, W], F32)
                nc.vector.tensor_scalar(
                    out=nb_yx,
                    in0=lnz,
                    scalar1=lng,
                    scalar2=-1.0,
                    op0=ALU.subtract,
                    op1=ALU.mult,
                )
                # hi/lo fp16 split, bounced through DRAM into the two
                # augmented q partitions: e' = e + nb_hi + nb_lo
                nbh = stats.tile([128, W], F16)
                nc.vector.tensor_copy(nbh, nb_yx)
                nbh32 = stats.tile([128, W], F32)
                nc.vector.tensor_copy(nbh32, nbh)
                nbl = stats.tile([128, W], F16)
                nc.vector.tensor_tensor(nbl, nb_yx, nbh32, ALU.subtract)
                nc.sync.dma_start(
                    out=nb_scr[0:1, :].rearrange("o (y x) -> (o y) x", x=W),
                    in_=nbh,
                )
                nc.sync.dma_start(
                    out=nb_scr[1:2, :].rearrange("o (y x) -> (o y) x", x=W),
                    in_=nbl,
                )
                nc.sync.dma_start(
                    out=q_sb[CQK : CQK + 2, :, :].rearrange(
                        "c y x -> c (y x)"
                    ),
                    in_=nb_scr[:, :],
                )

                # ---- a passes: swapped operands (k stationary, q moving)
                # so psum comes out [key, query]; exp writes maps directly.
                # col pass first so P3's col branch can start earliest.
                for x0 in range(0, W, 4):
                    e_ps = p2ps.tile([128, 4, 128], F32, tag="e_ps", bufs=4)
                    for j in range(4):
                        nc.tensor.matmul(
                            e_ps[:, j, :],
                            k_sb[:, :, x0 + j],
                            q_sb[:, :, x0 + j],
                            start=True,
                            stop=True,
                        )
                    nc.scalar.activation(
                        a_colT_t[:, x0 : x0 + 4, :].rearrange(
                            "p a b -> p (a b)"
                        ),
                        e_ps.rearrange("p a b -> p (a b)"),
                        AF.Exp,
                    )
                    # zero the self-pixel (diag g == yq) in the stored map
                    nc.gpsimd.affine_select(
                        out=a_colT_t[:, x0 : x0 + 4, :],
                        in_=a_colT_t[:, x0 : x0 + 4, :],
                        compare_op=ALU.not_equal,
                        fill=0.0,
                        base=0,
                        pattern=[[0, 4], [-1, 128]],
                        channel_multiplier=1,
                    )
                for y0 in range(0, H, 4):
                    e_ps = p2ps.tile([128, 4, 128], F32, tag="e_ps", bufs=4)
                    for j in range(4):
                        nc.tensor.matmul(
                            e_ps[:, j, :],
                            k_sb[:, y0 + j, :],
                            q_sb[:, y0 + j, :],
                            start=True,
                            stop=True,
                        )
                    nc.scalar.activation(
                        a_rowT_t[:, y0 : y0 + 4, :].rearrange(
                            "p a b -> p (a b)"
                        ),
                        e_ps.rearrange("p a b -> p (a b)"),
                        AF.Exp,
                    )

            if dbg:
                for name, src in (
                    ("dq", q_sb),
                    ("dk", k_sb),
                    ("ds1", s1),
                    ("ds2", s2),
                    ("dart", a_rowT_t),
                    ("dact", a_colT_t),
                ):
                    d = dbg_outs[name]
                    nc.sync.dma_start(
                        out=d[...].rearrange("a b c -> a (b c)")
                        if len(d.shape) == 3
                        else d[...],
                        in_=src.rearrange("p a b -> p (a b)")
                        if len(src.shape) == 3
                        else src[0 : d.shape[0], :],
                    )

        # ==================================================================
        # P3: attention application, oc-pair at a time
        # ==================================================================
        with ExitStack() as p3:
            accp = p3.enter_context(tc.tile_pool(name="accp", bufs=1))
            vcolp = p3.enter_context(tc.tile_pool(name="vcolp", bufs=8))
            vrowp = p3.enter_context(tc.tile_pool(name="vrowp", bufs=8))
            xres = p3.enter_context(tc.tile_pool(name="xres", bufs=6))
            outp = p3.enter_context(tc.tile_pool(name="outp", bufs=4))

            with tc.tile_pool(name="p3ps", bufs=1, space="PSUM") as p3ps:
                for op in range(OC // 2):  # oc pair
                    oc0 = 2 * op
                    # --- col branch: acc[c', x, y] per oc in pair --------
                    accs = [
                        accp.tile(
                            [128, W, H], F16, tag=f"acc{s}", name=f"acc_{op}_{s}"
                        )
                        for s in range(2)
                    ]
                    for x0 in range(0, W, 4):
                        vc = vcolp.tile([128, 4, 256], F16, tag="vc")
                        nc.sync.dma_start(
                            out=vc,
                            in_=_dap(
                                v_scr,
                                x0 * C + oc0 * 128,
                                [(W * C, 128), (C, 4), (1, 256)],
                            ),
                        )
                        for s in range(2):
                            pc_ps = p3ps.tile(
                                [128, 4, 128], F32, tag="pc", bufs=4
                            )
                            for j in range(4):
                                nc.tensor.matmul(
                                    pc_ps[:, j, :],
                                    vc[:, j, s * 128 : (s + 1) * 128],
                                    a_colT_t[:, x0 + j, :],
                                    start=True,
                                    stop=True,
                                )
                            ccp = (
                                nc.scalar.copy
                                if (x0 // 4 + s) % 2 == 0
                                else nc.vector.tensor_copy
                            )
                            ccp(
                                accs[s][:, x0 : x0 + 4, :].rearrange(
                                    "c x y -> c (x y)"
                                ),
                                pc_ps.rearrange("c x y -> c (x y)"),
                            )

                    if dbg and op == 0:
                        nc.sync.dma_start(
                            out=dbg_outs["dacc0"][...].rearrange(
                                "a b c -> a (b c)"
                            ),
                            in_=accs[0].rearrange("p a b -> p (a b)"),
                        )
                    # --- row branch + combine + residual -----------------
                    for y0 in range(0, H, 4):
                        vr = vrowp.tile([128, 4, 256], F16, tag="vr")
                        nc.sync.dma_start(
                            out=vr,
                            in_=_dap(
                                v_scr,
                                y0 * W * C + oc0 * 128,
                                [(C, 128), (W * C, 4), (1, 256)],
                            ),
                        )
                        for s in range(2):
                            oc = oc0 + s
                            xr = xres.tile([128, 4, 128], F32, tag="xr")
                            nc.gpsimd.dma_start(
                                out=xr.rearrange("c r w -> c (r w)"),
                                in_=x_d[
                                    oc * 128 : (oc + 1) * 128, y0 : y0 + 4, :
                                ].rearrange("c r w -> c (r w)"),
                            )
                            pr_ps = p3ps.tile(
                                [128, 4, 128], F32, tag="pr", bufs=4
                            )
                            # col acc relabel (x<->y) first: whole-tile
                            # identity matmul with strided moving operand
                            acc_ap = accs[s][...]
                            acc_mov = bass.AP(
                                tensor=acc_ap.tensor,
                                offset=acc_ap.offset + y0,
                                ap=[list(acc_ap.ap[0]), [1, 4], [H, W]],
                            )
                            nc.tensor.matmul(
                                pr_ps.rearrange("c r w -> c (r w)"),
                                ident16,
                                acc_mov,
                                start=True,
                                stop=False,
                                skip_group_check=True,
                            )
                            for j in range(4):
                                nc.tensor.matmul(
                                    pr_ps[:, j, :],
                                    vr[:, j, s * 128 : (s + 1) * 128],
                                    a_rowT_t[:, y0 + j, :],
                                    start=False,
                                    stop=(j == 3),
                                    skip_group_check=True,
                                )
                            ot = outp.tile([128, 4, 128], F16, tag="ot")
                            nc.vector.scalar_tensor_tensor(
                                out=ot.rearrange("c r w -> c (r w)"),
                                in0=pr_ps.rearrange("c r w -> c (r w)"),
                                scalar=gbv_sb[:, oc : oc + 1],
                                in1=xr.rearrange("c r w -> c (r w)"),
                                op0=ALU.add,
                                op1=ALU.add,
                            )
                            nc.sync.dma_start(
                                out=out_d[
                                    oc * 128 : (oc + 1) * 128, y0 : y0 + 4, :
                                ].rearrange("c r w -> c (r w)"),
                                in_=ot.rearrange("p a b -> p (a b)"),
                            )

    nc.finalize()
    return nc


_NC_CACHE = {}


def _get_nc():
    if "nc" not in _NC_CACHE:
        _NC_CACHE["nc"] = build()
    return _NC_CACHE["nc"]


def kernel(**inputs) -> np.ndarray:
    x = np.ascontiguousarray(np.asarray(inputs["x"], dtype=np.float32))
    n = x.shape[0]
    assert x.shape == (n, C, H, W)
    shared = {
        name: np.ascontiguousarray(np.asarray(inputs[name], dtype=np.float32))
        for name in ("Wq", "bq", "Wk", "bk", "Wv", "bv", "gamma")
    }
    nc = _get_nc()
    in_maps = [{"x": x[i], **shared} for i in range(n)]
    res = run_bass_kernel_spmd(nc, in_maps, core_ids=list(range(n)))
    return np.stack(
        [res.results[i]["out"].astype(np.float32) for i in range(n)], axis=0
    )


if __name__ == "__main__":
    rng = np.random.default_rng(0)
    demo = {
        "x": rng.standard_normal((N_CORES, C, H, W), dtype=np.float32),
        "Wq": rng.standard_normal((CQK, C), dtype=np.float32) / np.sqrt(C),
        "bq": np.zeros(CQK, np.float32),
        "Wk": rng.standard_normal((CQK, C), dtype=np.float32) / np.sqrt(C),
        "bk": np.zeros(CQK, np.float32),
        "Wv": rng.standard_normal((C, C), dtype=np.float32) / np.sqrt(C),
        "bv": np.zeros(C, np.float32),
        "gamma": np.ones(1, np.float32),
    }
    out = kernel(**demo)
    print("out", out.shape, out.dtype, np.abs(out).mean())


# revision 26
# speedup vs baseline: 1.7946x; 1.0397x over previous
"""Criss-Cross Attention (CCA) Trainium2 Bass kernel — v2.

Problem: n=8 images of (c=512, h=128, w=128); per-pixel projections
q,k (64ch) and v (512ch); row + column attention with joint softmax over
the 256 (w + h) logits per pixel (self pixel masked out of the column
branch); out = gamma * att + x.

Sharding: data-parallel over batch — one image per NeuronCore (8 cores).

v2 design notes (all transposes eliminated):
  P1: stream x in 4-row blocks; project q,k channel-major into SBUF;
      project v PIXEL-major (x16 row-slice as matmul stationary, WvT as
      moving) giving [x, c512] tiles, stored to DRAM as v_scr[y][x][c].
  P2: sum pass (query-on-partition e-matmuls, exp, reduce) -> Z;
      nb = -(lnZ - ln gamma) folded into fp16 hi/lo aug rows of q_sb.
      a-pass with SWAPPED operands (k stationary, q moving) so the
      attention maps come out KEY-on-partition and are written straight
      to SBUF — no XBAR transpose DMAs.
  P3: per oc-pair: col branch (v column tiles read from v_scr as 512B
      lines, v-stationary matmuls) -> contiguous fp16 acc [c,x,y];
      row branch matmuls accumulate into PSUM, the col acc is folded in
      by an identity-matmul whose strided MOVING operand does the
      (x<->y) relabel for free on the PE; one fused DVE op adds the
      residual x and gamma*bv, then a straight DMA writes out.
"""

import sys

for _p in ("/opt/trn_rl_repo",):
    if _p not in sys.path:
        sys.path.insert(0, _p)

from contextlib import ExitStack

import numpy as np

from concourse import bacc
import concourse.bass as bass
import concourse.mybir as mybir
import concourse.tile as tile
from concourse.bass_utils import run_bass_kernel_spmd

F32 = mybir.dt.float32
F16 = mybir.dt.float16
AX = mybir.AxisListType
ALU = mybir.AluOpType
AF = mybir.ActivationFunctionType

N_CORES = 8
C, H, W = 512, 128, 128
CQK = 64
KC = 4  # input-channel chunks of 128
OC = 4  # output-channel chunks of 128
NEG_INF = -1e9


def _dap(t, offset, dims):
    """Raw DRAM access pattern: dims = [(stride, count), ...] in elements."""
    a = t[...]
    return bass.AP(
        tensor=a.tensor, offset=a.offset + offset, ap=[[s, n] for s, n in dims]
    )


def build(n_cores: int = N_CORES, dbg: bool = False):
    nc = bacc.Bacc("TRN2", debug=False, num_devices=n_cores)

    x_d = nc.dram_tensor("x", [C, H, W], F32, kind="ExternalInput")
    wq_d = nc.dram_tensor("Wq", [CQK, C], F32, kind="ExternalInput")
    bq_d = nc.dram_tensor("bq", [CQK], F32, kind="ExternalInput")
    wk_d = nc.dram_tensor("Wk", [CQK, C], F32, kind="ExternalInput")
    bk_d = nc.dram_tensor("bk", [CQK], F32, kind="ExternalInput")
    wv_d = nc.dram_tensor("Wv", [C, C], F32, kind="ExternalInput")
    bv_d = nc.dram_tensor("bv", [C], F32, kind="ExternalInput")
    g_d = nc.dram_tensor("gamma", [1], F32, kind="ExternalInput")
    out_d = nc.dram_tensor("out", [C, H, W], F16, kind="ExternalOutput")

    v_scr = nc.dram_tensor(
        "v_scr", [H, W, C], F16, kind="ExternalOutput" if dbg else "Internal"
    )  # pixel-major
    nb_scr = nc.dram_tensor("nb_scr", [2, H * W], F16)
    if dbg:
        dbg_outs = {
            "dq": nc.dram_tensor("dq", [CQK + 2, H, W], F16, kind="ExternalOutput"),
            "dk": nc.dram_tensor("dk", [CQK + 2, H, W], F16, kind="ExternalOutput"),
            "ds1": nc.dram_tensor("ds1", [128, H], F32, kind="ExternalOutput"),
            "ds2": nc.dram_tensor("ds2", [128, W], F32, kind="ExternalOutput"),
            "dart": nc.dram_tensor("dart", [128, H, 128], F16, kind="ExternalOutput"),
            "dact": nc.dram_tensor("dact", [128, W, 128], F16, kind="ExternalOutput"),
            "dacc0": nc.dram_tensor("dacc0", [128, W, H], F16, kind="ExternalOutput"),
        }

    with tile.TileContext(nc) as tc, ExitStack() as ctx:
        const = ctx.enter_context(tc.tile_pool(name="const", bufs=1))
        stats = ctx.enter_context(tc.tile_pool(name="stats", bufs=1))

        # ---- constants ----------------------------------------------------
        ident32 = const.tile([128, 128], F32)
        from concourse.masks import make_identity

        make_identity(nc, ident32)
        ident16 = const.tile([128, 128], F16)
        nc.vector.tensor_copy(ident16, ident32)



        bq_sb = const.tile([CQK, 1], F32)
        nc.sync.dma_start(out=bq_sb, in_=bq_d[:].rearrange("(a b) -> a b", b=1))
        bk_sb = const.tile([CQK, 1], F32)
        nc.sync.dma_start(out=bk_sb, in_=bk_d[:].rearrange("(a b) -> a b", b=1))
        bv_sb = const.tile([128, OC], F32)
        nc.sync.dma_start(
            out=bv_sb, in_=bv_d[:].rearrange("(o p) -> p o", p=128)
        )
        g_ap = g_d[:]
        g_bcast = bass.AP(
            tensor=g_ap.tensor, offset=g_ap.offset, ap=[[0, 128], [1, 1]]
        )
        g_sb = const.tile([128, 1], F32)
        nc.gpsimd.dma_start(out=g_sb, in_=g_bcast)
        lng = stats.tile([128, 1], F32)
        nc.scalar.activation(lng, g_sb, AF.Ln)
        # gamma * bv, added in the final residual op
        gbv_sb = const.tile([128, OC], F32)
        nc.vector.tensor_scalar(
            out=gbv_sb, in0=bv_sb, scalar1=g_sb, scalar2=None, op0=ALU.mult
        )

        # transposed projection weights (fp16): wqkT [128, KC, 128] where
        # columns 0:64 = Wq^T chunk, 64:128 = Wk^T chunk; wvT [128, KC, 512]
        wqkT = const.tile([128, KC, 128], F16)
        wvT = const.tile([128, KC, C], F16)
        with tc.tile_pool(name="wprep", bufs=2) as wprep, tc.tile_pool(
            name="wps", bufs=2, space="PSUM"
        ) as wps:
            for kc in range(KC):
                for w_d, col0 in ((wq_d, 0), (wk_d, CQK)):
                    raw = wprep.tile([CQK, 128], F32, tag="rawqk")
                    nc.sync.dma_start(
                        out=raw, in_=w_d[:, kc * 128 : (kc + 1) * 128]
                    )
                    tps = wps.tile([128, CQK], F32, tag="tqk")
                    nc.tensor.transpose(tps, raw, ident32[:CQK, :CQK])
                    nc.vector.tensor_copy(
                        wqkT[:, kc, col0 : col0 + CQK], tps
                    )
                for oc in range(OC):
                    rawv = wprep.tile([128, 128], F32, tag="rawv")
                    nc.sync.dma_start(
                        out=rawv,
                        in_=wv_d[
                            oc * 128 : (oc + 1) * 128, kc * 128 : (kc + 1) * 128
                        ],
                    )
                    tps2 = wps.tile([128, 128], F32, tag="tv")
                    nc.tensor.transpose(tps2, rawv, ident32)
                    nc.vector.tensor_copy(
                        wvT[:, kc, oc * 128 : (oc + 1) * 128], tps2
                    )

        # ---- persistent attention maps (key-on-partition, fp16) ----------
        a_rowT = ctx.enter_context(tc.tile_pool(name="a_rowT", bufs=1))
        a_colT = ctx.enter_context(tc.tile_pool(name="a_colT", bufs=1))
        a_rowT_t = a_rowT.tile([128, H, 128], F16)  # (xk, y, xq)
        a_colT_t = a_colT.tile([128, W, 128], F16)  # (g,  x, yq)

        s1 = stats.tile([128, H], F32)  # [xq, y] row-branch exp sums
        s2 = stats.tile([128, W], F32)  # [yq, x] col-branch exp sums

        # ==================================================================
        # P1 + P2 in a nested scope so q/k free their SBUF before P3
        # ==================================================================
        with ExitStack() as p12:
            qk = p12.enter_context(tc.tile_pool(name="qk", bufs=1))
            # rows 0:64 = channels; rows 64,65 = nb hi/lo (q) and ones (k)
            q_sb = qk.tile([CQK + 2, H, W], F16)  # (c, y, x)
            k_sb = qk.tile([CQK + 2, H, W], F16)
            nc.gpsimd.memset(q_sb[CQK : CQK + 2, :, :], 0.0)
            nc.gpsimd.memset(k_sb[CQK : CQK + 2, :, :], 1.0)

            # ---------------- P1: projections -----------------------------
            with tc.tile_pool(name="xin", bufs=3) as xin, tc.tile_pool(
                name="x16", bufs=3
            ) as x16p, tc.tile_pool(name="v16", bufs=2) as v16p, tc.tile_pool(
                name="p1ps", bufs=1, space="PSUM"
            ) as p1ps:
                for b in range(H // 4):
                    y0 = 4 * b
                    xt = xin.tile([128, KC, 512], F32, tag="xt")
                    for kc in range(KC):
                        nc.sync.dma_start(
                            out=xt[:, kc, :],
                            in_=x_d[
                                kc * 128 : (kc + 1) * 128, y0 : y0 + 4, :
                            ].rearrange("c r w -> c (r w)"),
                        )
                    x16 = x16p.tile([128, KC, 512], F16, tag="x16")
                    cast = nc.scalar.copy if b % 2 == 0 else nc.vector.tensor_copy
                    cast(
                        x16.rearrange("c k w -> c (k w)"),
                        xt.rearrange("c k w -> c (k w)"),
                    )

                    # q,k channel-major: psum [qk128, (4y,128x)]
                    qk_ps = p1ps.tile([128, 512], F32, tag="qkps", bufs=2)
                    for kc in range(KC):
                        nc.tensor.matmul(
                            qk_ps,
                            wqkT[:, kc, :],
                            x16[:, kc, :],
                            start=(kc == 0),
                            stop=(kc == KC - 1),
                        )
                    nc.vector.tensor_scalar_add(
                        q_sb[0:CQK, y0 : y0 + 4, :].rearrange(
                            "c r w -> c (r w)"
                        ),
                        qk_ps[0:CQK, :],
                        bq_sb,
                    )
                    nc.vector.tensor_scalar_add(
                        k_sb[0:CQK, y0 : y0 + 4, :].rearrange(
                            "c r w -> c (r w)"
                        ),
                        qk_ps[CQK:128, :],
                        bk_sb,
                    )

                    # v pixel-major: per row y, psum [x, c512]
                    v16 = v16p.tile([128, 4, C], F16, tag="v16")
                    for j in range(4):
                        v_ps = p1ps.tile([128, C], F32, tag="vps", bufs=4)
                        for kc in range(KC):
                            nc.tensor.matmul(
                                v_ps,
                                x16[:, kc, j * 128 : (j + 1) * 128],
                                wvT[:, kc, :],
                                start=(kc == 0),
                                stop=(kc == KC - 1),
                            )
                        vcp = (
                            nc.scalar.copy if j % 2 == 0 else nc.vector.tensor_copy
                        )
                        vcp(v16[:, j, :], v_ps)
                    nc.sync.dma_start(
                        out=_dap(
                            v_scr,
                            y0 * W * C,
                            [(C, 128), (W * C, 4), (1, C)],
                        ),
                        in_=v16,
                    )

            # ---------------- P2: softmax statistics ----------------------
            trash = p12.enter_context(tc.tile_pool(name="trash", bufs=4))

            with tc.tile_pool(name="p2ps", bufs=1, space="PSUM") as p2ps:
                # ---- sum pass (query on partitions; q aug rows still 0) --
                for y0 in range(0, H, 4):
                    e_ps = p2ps.tile([128, 4, 128], F32, tag="e_ps", bufs=4)
                    for j in range(4):
                        nc.tensor.matmul(
                            e_ps[:, j, :],
                            q_sb[:, y0 + j, :],
                            k_sb[:, y0 + j, :],
                            start=True,
                            stop=True,
                        )
                    tr = trash.tile([128, 4, 128], F32, tag="trash")
                    nc.scalar.activation(
                        tr.rearrange("p a b -> p (a b)"),
                        e_ps.rearrange("p a b -> p (a b)"),
                        AF.Exp,
                    )
                    nc.vector.reduce_sum(s1[:, y0 : y0 + 4], tr, axis=AX.X)
                for x0 in range(0, W, 4):
                    e_ps = p2ps.tile([128, 4, 128], F32, tag="e_ps", bufs=4)
                    for j in range(4):
                        nc.tensor.matmul(
                            e_ps[:, j, :],
                            q_sb[:, :, x0 + j],
                            k_sb[:, :, x0 + j],
                            start=True,
                            stop=True,
                        )
                    tr = trash.tile([128, 4, 128], F32, tag="trash")
                    nc.scalar.activation(
                        tr.rearrange("p a b -> p (a b)"),
                        e_ps.rearrange("p a b -> p (a b)"),
                        AF.Exp,
                    )
                    # zero the self-pixel (diag yk == yq) before the reduce
                    nc.gpsimd.affine_select(
                        out=tr,
                        in_=tr,
                        compare_op=ALU.not_equal,
                        fill=0.0,
                        base=0,
                        pattern=[[0, 4], [-1, 128]],
                        channel_multiplier=1,
                    )
                    nc.vector.reduce_sum(s2[:, x0 : x0 + 4], tr, axis=AX.X)

                # ---- nb[y,x] = -(ln(Z) - ln(gamma)); ln via exponent
                # extraction so any fp32 Z is in the ACT Ln table range ----
                zt_ps = p2ps.tile([128, 128], F32, tag="zt", bufs=1)
                nc.tensor.transpose(zt_ps, s1, ident32)
                z_yx = stats.tile([128, W], F32)
                nc.vector.tensor_tensor(z_yx, zt_ps, s2, ALU.add)
                z_i = z_yx[...].bitcast(mybir.dt.int32)
                e_i32 = stats.tile([128, W], mybir.dt.int32)
                nc.vector.tensor_scalar(
                    out=e_i32,
                    in0=z_i,
                    scalar1=23,
                    scalar2=None,
                    op0=ALU.logical_shift_right,
                )
                ef = stats.tile([128, W], F32)
                nc.vector.tensor_scalar(
                    out=ef,
                    in0=e_i32,
                    scalar1=127,
                    scalar2=None,
                    op0=ALU.subtract,
                )
                mant = stats.tile([128, W], F32)
                nc.vector.tensor_scalar(
                    out=mant[...].bitcast(mybir.dt.int32),
                    in0=z_i,
                    scalar1=0x007FFFFF,
                    scalar2=0x3F800000,
                    op0=ALU.bitwise_and,
                    op1=ALU.bitwise_or,
                )
                lnm = stats.tile([128, W], F32)
                nc.scalar.activation(lnm, mant, AF.Ln)
                lnz = stats.tile([128, W], F32)
                nc.vector.scalar_tensor_tensor(
                    out=lnz,
                    in0=ef,
                    scalar=float(np.log(2.0)),
                    in1=lnm,
                    op0=ALU.mult,
                    op1=ALU.add,
                )
                nb_yx = stats.tile([128, W], F32)
                nc.vector.tensor_scalar(
                    out=nb_yx,
                    in0=lnz,
                    scalar1=lng,
                    scalar2=-1.0,
                    op0=ALU.subtract,
                    op1=ALU.mult,
                )
                # hi/lo fp16 split, bounced through DRAM into the two
                # augmented q partitions: e' = e + nb_hi + nb_lo
                nbh = stats.tile([128, W], F16)
                nc.vector.tensor_copy(nbh, nb_yx)
                nbh32 = stats.tile([128, W], F32)
                nc.vector.tensor_copy(nbh32, nbh)
                nbl = stats.tile([128, W], F16)
                nc.vector.tensor_tensor(nbl, nb_yx, nbh32, ALU.subtract)
                nc.sync.dma_start(
                    out=nb_scr[0:1, :].rearrange("o (y x) -> (o y) x", x=W),
                    in_=nbh,
                )
                nc.sync.dma_start(
                    out=nb_scr[1:2, :].rearrange("o (y x) -> (o y) x", x=W),
                    in_=nbl,
                )
                nc.sync.dma_start(
                    out=q_sb[CQK : CQK + 2, :, :].rearrange(
                        "c y x -> c (y x)"
                    ),
                    in_=nb_scr[:, :],
                )

                # ---- a passes: swapped operands (k stationary, q moving)
                # so psum comes out [key, query]; exp writes maps directly.
                # col pass first so P3's col branch can start earliest.
                for x0 in range(0, W, 4):
                    e_ps = p2ps.tile([128, 4, 128], F32, tag="e_ps", bufs=4)
                    for j in range(4):
                        nc.tensor.matmul(
                            e_ps[:, j, :],
                            k_sb[:, :, x0 + j],
                            q_sb[:, :, x0 + j],
                            start=True,
                            stop=True,
                        )
                    nc.scalar.activation(
                        a_colT_t[:, x0 : x0 + 4, :].rearrange(
                            "p a b -> p (a b)"
                        ),
                        e_ps.rearrange("p a b -> p (a b)"),
                        AF.Exp,
                    )
                    # zero the self-pixel (diag g == yq) in the stored map
                    nc.gpsimd.affine_select(
                        out=a_colT_t[:, x0 : x0 + 4, :],
                        in_=a_colT_t[:, x0 : x0 + 4, :],
                        compare_op=ALU.not_equal,
                        fill=0.0,
                        base=0,
                        pattern=[[0, 4], [-1, 128]],
                        channel_multiplier=1,
                    )
                for y0 in range(0, H, 4):
                    e_ps = p2ps.tile([128, 4, 128], F32, tag="e_ps", bufs=4)
                    for j in range(4):
                        nc.tensor.matmul(
                            e_ps[:, j, :],
                            k_sb[:, y0 + j, :],
                            q_sb[:, y0 + j, :],
                            start=True,
                            stop=True,
                        )
                    nc.scalar.activation(
                        a_rowT_t[:, y0 : y0 + 4, :].rearrange(
                            "p a b -> p (a b)"
                        ),
                        e_ps.rearrange("p a b -> p (a b)"),
                        AF.Exp,
                    )

            if dbg:
                for name, src in (
                    ("dq", q_sb),
                    ("dk", k_sb),
                    ("ds1", s1),
                    ("ds2", s2),
                    ("dart", a_rowT_t),
                    ("dact", a_colT_t),
                ):
                    d = dbg_outs[name]
                    nc.sync.dma_start(
                        out=d[...].rearrange("a b c -> a (b c)")
                        if len(d.shape) == 3
                        else d[...],
                        in_=src.rearrange("p a b -> p (a b)")
                        if len(src.shape) == 3
                        else src[0 : d.shape[0], :],
                    )

        # ==================================================================
        # P3: attention application, oc-pair at a time
        # ==================================================================
        with ExitStack() as p3:
            accp = p3.enter_context(tc.tile_pool(name="accp", bufs=1))
            vcolp = p3.enter_context(tc.tile_pool(name="vcolp", bufs=8))
            vrowp = p3.enter_context(tc.tile_pool(name="vrowp", bufs=8))
            xres = p3.enter_context(tc.tile_pool(name="xres", bufs=6))
            outp = p3.enter_context(tc.tile_pool(name="outp", bufs=4))

            with tc.tile_pool(name="p3ps", bufs=1, space="PSUM") as p3ps:
                for op in range(OC // 2):  # oc pair
                    oc0 = 2 * op
                    # --- col branch: acc[c', x, y] per oc in pair --------
                    accs = [
                        accp.tile(
                            [128, W, H], F16, tag=f"acc{s}", name=f"acc_{op}_{s}"
                        )
                        for s in range(2)
                    ]
                    for x0 in range(0, W, 4):
                        vc = vcolp.tile([128, 4, 256], F16, tag="vc")
                        nc.sync.dma_start(
                            out=vc,
                            in_=_dap(
                                v_scr,
                                x0 * C + oc0 * 128,
                                [(W * C, 128), (C, 4), (1, 256)],
                            ),
                        )
                        for s in range(2):
                            pc_ps = p3ps.tile(
                                [128, 4, 128], F32, tag="pc", bufs=4
                            )
                            for j in range(4):
                                nc.tensor.matmul(
                                    pc_ps[:, j, :],
                                    vc[:, j, s * 128 : (s + 1) * 128],
                                    a_colT_t[:, x0 + j, :],
                                    start=True,
                                    stop=True,
                                )
                            ccp = (
                                nc.scalar.copy
                                if (x0 // 4 + s) % 2 == 0
                                else nc.vector.tensor_copy
                            )
                            ccp(
                                accs[s][:, x0 : x0 + 4, :].rearrange(
                                    "c x y -> c (x y)"
                                ),
                                pc_ps.rearrange("c x y -> c (x y)"),
                            )

                    if dbg and op == 0:
                        nc.sync.dma_start(
                            out=dbg_outs["dacc0"][...].rearrange(
                                "a b c -> a (b c)"
                            ),
                            in_=accs[0].rearrange("p a b -> p (a b)"),
                        )
                    # --- row branch + combine + residual -----------------
                    # supergroups of 2 y4-blocks x 2 oc: the 4 fold matmuls
                    # run back-to-back (identity stationary loaded once)
                    for yg in range(0, H, 8):
                        vrs = []
                        for g in range(2):
                            y0 = yg + 4 * g
                            vr = vrowp.tile([128, 4, 256], F16, tag="vr")
                            nc.sync.dma_start(
                                out=vr,
                                in_=_dap(
                                    v_scr,
                                    y0 * W * C + oc0 * 128,
                                    [(C, 128), (W * C, 4), (1, 256)],
                                ),
                            )
                            vrs.append(vr)
                        xrs = {}
                        for s in range(2):
                            oc = oc0 + s
                            for g in range(2):
                                y0 = yg + 4 * g
                                xr = xres.tile([128, 4, 128], F32, tag="xr")
                                nc.gpsimd.dma_start(
                                    out=xr.rearrange("c r w -> c (r w)"),
                                    in_=x_d[
                                        oc * 128 : (oc + 1) * 128,
                                        y0 : y0 + 4,
                                        :,
                                    ].rearrange("c r w -> c (r w)"),
                                )
                                xrs[(s, g)] = xr
                        prs = {}
                        for s in range(2):
                            for g in range(2):
                                y0 = yg + 4 * g
                                pr_ps = p3ps.tile(
                                    [128, 4, 128], F32, tag="pr", bufs=4,
                                    name=f"pr_{op}_{yg}_{s}_{g}",
                                )
                                acc_ap = accs[s][...]
                                acc_mov = bass.AP(
                                    tensor=acc_ap.tensor,
                                    offset=acc_ap.offset + y0,
                                    ap=[list(acc_ap.ap[0]), [1, 4], [H, W]],
                                )
                                nc.tensor.matmul(
                                    pr_ps.rearrange("c r w -> c (r w)"),
                                    ident16,
                                    acc_mov,
                                    start=True,
                                    stop=False,
                                    skip_group_check=True,
                                )
                                prs[(s, g)] = pr_ps
                        for s in range(2):
                            for g in range(2):
                                y0 = yg + 4 * g
                                pr_ps = prs[(s, g)]
                                for j in range(4):
                                    nc.tensor.matmul(
                                        pr_ps[:, j, :],
                                        vrs[g][:, j, s * 128 : (s + 1) * 128],
                                        a_rowT_t[:, y0 + j, :],
                                        start=False,
                                        stop=(j == 3),
                                        skip_group_check=True,
                                    )
                        for s in range(2):
                            oc = oc0 + s
                            for g in range(2):
                                y0 = yg + 4 * g
                                ot = outp.tile([128, 4, 128], F16, tag="ot")
                                nc.vector.scalar_tensor_tensor(
                                    out=ot.rearrange("c r w -> c (r w)"),
                                    in0=prs[(s, g)].rearrange(
                                        "c r w -> c (r w)"
                                    ),
                                    scalar=gbv_sb[:, oc : oc + 1],
                                    in1=xrs[(s, g)].rearrange(
                                        "c r w -> c (r w)"
                                    ),
                                    op0=ALU.add,
                                    op1=ALU.add,
                                )
                                nc.sync.dma_start(
                                    out=out_d[
                                        oc * 128 : (oc + 1) * 128,
                                        y0 : y0 + 4,
                                        :,
                                    ].rearrange("c r w -> c (r w)"),
                                    in_=ot.rearrange("p a b -> p (a b)"),
                                )

    nc.finalize()
    return nc


_NC_CACHE = {}


def _get_nc():
    if "nc" not in _NC_CACHE:
        _NC_CACHE["nc"] = build()
    return _NC_CACHE["nc"]


def kernel(**inputs) -> np.ndarray:
    x = np.ascontiguousarray(np.asarray(inputs["x"], dtype=np.float32))
    n = x.shape[0]
    assert x.shape == (n, C, H, W)
    shared = {
        name: np.ascontiguousarray(np.asarray(inputs[name], dtype=np.float32))
        for name in ("Wq", "bq", "Wk", "bk", "Wv", "bv", "gamma")
    }
    nc = _get_nc()
    in_maps = [{"x": x[i], **shared} for i in range(n)]
    res = run_bass_kernel_spmd(nc, in_maps, core_ids=list(range(n)))
    return np.stack(
        [res.results[i]["out"].astype(np.float32) for i in range(n)], axis=0
    )


if __name__ == "__main__":
    rng = np.random.default_rng(0)
    demo = {
        "x": rng.standard_normal((N_CORES, C, H, W), dtype=np.float32),
        "Wq": rng.standard_normal((CQK, C), dtype=np.float32) / np.sqrt(C),
        "bq": np.zeros(CQK, np.float32),
        "Wk": rng.standard_normal((CQK, C), dtype=np.float32) / np.sqrt(C),
        "bk": np.zeros(CQK, np.float32),
        "Wv": rng.standard_normal((C, C), dtype=np.float32) / np.sqrt(C),
        "bv": np.zeros(C, np.float32),
        "gamma": np.ones(1, np.float32),
    }
    out = kernel(**demo)
    print("out", out.shape, out.dtype, np.abs(out).mean())
